# revision 34
# baseline (speedup 1.0000x reference)
"""Trainium2 Bass kernel for EnetGnn (gnn_message_passing).

Data-parallel over batch N=8, one sample per NeuronCore. Per-core design:

1. Median pool: host stages negated fp16 blocks in [16, 128, 4, 64] tiles so
   each load is one contiguous 64KB DMA. DVE max8/match_replace rank-32
   rounds; medians flattened via two half PE transposes + DMA so the x
   channel stages while the second half of the median still runs.
2. KNN threshold: e'[i,j] = 2p_i.p_j - |p_j|^2 via K=4 fp16 matmuls into
   double-buffered 3-bank psum halves, ACT-evacuated to fp16 ef. Per-row
   16th-largest te via max8 + is_ge mask removal + max8 (all DVE).
3. S = Sign(ef - te + eps) on the ACT engine with per-row bias (no phase-2
   matmul recompute). S tiles land in one [128, 22, 2720] fp8 SBUF tensor
   with a ones-column for G.
4. All heavy matmul work (aggregation iters 1+2, q updates, g-MLP 2, conv)
   runs as one dense tail stream to keep the PE at its hot clock. The
   ones-column gives G; a rank-1 matmul broadcasts G so mts = G + S@gh
   evacuates at natural scale to fp16, making the q update all-fp16.
"""
import numpy as np
import concourse.bass as bass
import concourse.bacc as bacc
import concourse.mybir as mybir
import concourse.tile as tile
from concourse.bass_utils import run_bass_kernel_spmd

F32 = mybir.dt.float32
F16 = mybir.dt.float16
F8 = mybir.dt.float8e4
AF = mybir.ActivationFunctionType
ALU = mybir.AluOpType

N, C, H, W = 8, 128, 45, 60
HW = H * W                      # 2700
K = 16
NEG_F16 = -60000.0

CHUNKS6 = [(0, 512), (512, 512), (1024, 512), (1536, 512), (2048, 512), (2560, 140)]
AGG_CHUNKS = [(0, 512), (512, 512), (1024, 512), (1536, 512), (2048, 512), (2560, 141)]
PTILES = [(t * 128, 128) for t in range(21)] + [(2688, 12)]
RCHUNKS = [(0, 8), (8, 8), (16, 8), (24, 8), (32, 8), (40, 5)]
TGROUPS = [list(range(0, 8)), list(range(8, 16)), list(range(16, 22))]

_cache = {}


def _ensure_ntff_hook():
    import sys
    import types
    try:
        from antenv.axon_hooks import get_axon_ntff_profile_hook  # noqa: F401
        return
    except ImportError:
        pass
    try:
        mod = types.ModuleType("antenv.axon_hooks")
        mod._hook = None

        def set_axon_ntff_profile_hook(h):
            mod._hook = h

        def get_axon_ntff_profile_hook():
            return mod._hook

        mod.set_axon_ntff_profile_hook = set_axon_ntff_profile_hook
        mod.get_axon_ntff_profile_hook = get_axon_ntff_profile_hook
        sys.modules["antenv.axon_hooks"] = mod
        import antenv
        antenv.axon_hooks = mod
        from trn_agent_boot.trn_boot import _ntff_profile_via_ctypes
        hook = _ntff_profile_via_ctypes("/opt/axon/libaxon_pjrt.so")
        if hook is not None:
            mod.set_axon_ntff_profile_hook(hook)
    except Exception as e:  # profiling is best-effort
        print(f"ntff hook injection failed: {e}")


def _build(a0, a1, qa):
    nc = bacc.Bacc("TRN2", target_bir_lowering=False, debug=False, num_devices=8)

    h0_d = nc.dram_tensor("h0", (C, HW), F16, kind="ExternalInput")
    psrcb_d = nc.dram_tensor("psrcb", (16, 128, 4, 64), F16, kind="ExternalInput")
    gw0_d = nc.dram_tensor("gw0T", (C, C), F16, kind="ExternalInput")
    gw1_d = nc.dram_tensor("gw1T", (C, C), F16, kind="ExternalInput")
    qw1_d = nc.dram_tensor("qw1T", (C, C), F16, kind="ExternalInput")
    qw2_d = nc.dram_tensor("qw2T", (C, C), F16, kind="ExternalInput")
    cw_d = nc.dram_tensor("convwT", (C, 18, C), F16, kind="ExternalInput")
    bias_d = nc.dram_tensor("biases", (C, 4), F32, kind="ExternalInput")
    b1row_d = nc.dram_tensor("b1row", (2, C), F16, kind="ExternalInput")
    ident_d = nc.dram_tensor("ident", (C, C), F16, kind="ExternalInput")
    uvc_d = nc.dram_tensor("uvc", (2, 8, 2816), F16, kind="ExternalInput")
    out_d = nc.dram_tensor("out", (C, HW), F32, kind="ExternalOutput")

    with tile.TileContext(nc) as tc:
        with tc.tile_pool(name="sb", bufs=1) as sb, \
             tc.tile_pool(name="work", bufs=2) as work, \
             tc.tile_pool(name="ps", bufs=1, space="PSUM") as ps, \
             tc.tile_pool(name="dram", bufs=1, space="DRAM") as dram:

            projn_d = dram.tile([8192], F16, tag="projn_d")

            # median block DMAs first so the DVE phase starts immediately
            blks = []
            for g in range(16):
                blk = work.tile([128, 4, 64], F16, tag="blk", bufs=8,
                                name=f"blk_{g}")
                nc.sync.dma_start(blk[:], psrcb_d[g])
                blks.append(blk)

            # ---------------- persistent SBUF ----------------
            h0 = sb.tile([C, 2720], F16, tag="h0")
            nc.sync.dma_start(h0[:, 0:HW], h0_d[:])
            gw0 = sb.tile([C, C], F16, tag="gw0")
            nc.sync.dma_start(gw0[:], gw0_d[:])
            gw1 = sb.tile([C, C], F16, tag="gw1")
            nc.sync.dma_start(gw1[:], gw1_d[:])
            qw1 = sb.tile([C, C], F16, tag="qw1")
            nc.sync.dma_start(qw1[:], qw1_d[:])
            qw2 = sb.tile([C, C], F16, tag="qw2")
            nc.sync.dma_start(qw2[:], qw2_d[:])
            cw = sb.tile([C, 18, C], F16, tag="cw")
            nc.sync.dma_start(cw[:], cw_d[:])
            bia = sb.tile([C, 4], F32, tag="bias")
            nc.sync.dma_start(bia[:], bias_d[:])
            b1row = sb.tile([2, C], F16, tag="b1row")
            nc.sync.dma_start(b1row[:], b1row_d[:])
            ones1 = sb.tile([1, C], F16, tag="ones1")
            ident = sb.tile([C, C], F16, tag="ident")
            nc.sync.dma_start(ident[:], ident_d[:])

            U = sb.tile([8, 2816], F16, tag="U")       # [q; 1]
            nc.sync.dma_start(U[:], uvc_d[0])
            V = sb.tile([8, 2816], F16, tag="V")       # [q; -|p|^2/2]
            nc.sync.dma_start(V[:], uvc_d[1])
            Sbig = sb.tile([C, 22, 2720], F8, tag="Sbig")
            ghrm8 = sb.tile([C, 22, 128], F8, tag="ghrm8")
            M8 = sb.tile([C, 64, 8], F16, tag="M8")
            Mt = sb.tile([64, C], F16, tag="Mt")
            pad0 = sb.tile([C, H + 2, W + 2], F16, tag="pad0")
            pad1 = sb.tile([C, H + 2, W + 2], F16, tag="pad1")
            convacc = sb.tile([C, 2720], F32, tag="convacc")
            ones3 = sb.tile([3, 1], F16, tag="ones3")

            # memsets on gpsimd (DVE stays on the median path)
            nc.gpsimd.memset(Sbig[:, :, HW:HW + 1], 1.0)   # ones-cols for G
            nc.gpsimd.memset(pad0[:], 0.0)
            nc.gpsimd.memset(pad1[:], 0.0)
            nc.gpsimd.memset(ones3[:], 1.0)
            nc.gpsimd.memset(ones1[:], 1.0)

            # ---------------- median pooling + split flatten -----------------
            def median_range(glo, ghi):
                for g in range(glo, ghi):
                    blk = blks[g]
                    for s in range(4):
                        mm8 = work.tile([128, 8], F16, tag="mm8", bufs=8)
                        for rnd in range(3):
                            nc.vector.max(mm8[:], blk[:, s, :])
                            nc.vector.match_replace(blk[:, s, :], mm8[:],
                                                    blk[:, s, :], NEG_F16)
                        nc.vector.max(M8[:, g * 4 + s, :], blk[:, s, :])

            def flatten_half(half):
                lo, nc_ = (0, 32) if half == 0 else (32, 32)
                mtp = ps.tile([C, 1024], F16, tag="sm", bufs=2, name=f"mtp{half}")
                Mcols = M8[:, lo:lo + 32, 7:8].rearrange("p a b -> p (a b)")
                nc.tensor.transpose(mtp[0:32, 0:128], Mcols, ident[:])
                nc.scalar.activation(Mt[lo:lo + 32, :], mtp[0:32, 0:128], AF.Copy)
                projn_r = projn_d.rearrange("(a b) -> a b", b=128)
                nc.sync.dma_start(projn_r[lo:lo + 32, :], Mt[lo:lo + 32, :])

            median_range(0, 8)
            median_range(8, 16)

            # ---------------- iter-1 g-MLP + conv h0-half (under median) -----
            def mlp_layer(w, h_in, out, it, lab, bias, alpha):
                for half, o0, on in ((0, 0, 1536), (1, 1536, HW - 1536)):
                    gp = ps.tile([C, 1536], F32, tag="big3", bufs=2,
                                 name=f"{lab}_{it}_{half}")
                    for c0, ncn in (CHUNKS6[:3] if half == 0 else CHUNKS6[3:]):
                        nc.tensor.matmul(gp[:, c0 - o0:c0 - o0 + ncn], w[:],
                                         h_in[:, c0:c0 + ncn], start=True, stop=True)
                    nc.scalar.activation(out[:, o0:o0 + on], gp[:, 0:on], AF.Prelu,
                                         bias=bias, alpha=alpha)

            def gmlp_t(h_in, it):
                """g-MLP: layer 1 straight, layer 2 in transposed orientation
                writing gh2^T tiles directly into ghrm8 (fp8). The b1 bias is
                added via a rank-1 matmul (per-feature = free dim here)."""
                gh1 = work.tile([C, 2720], F16, tag="gh", bufs=1, name=f"gh1_{it}")
                mlp_layer(gw0, h_in, gh1, it, "g1", bia[:, 0:1], a0)
                for jt, (j0, nj) in enumerate(PTILES):
                    lp32 = ps.tile([C, 512], F32, tag="sm", bufs=2,
                                   name=f"l2t_{it}_{jt}")
                    nc.tensor.matmul(lp32[0:nj, 0:128], gh1[:, j0:j0 + nj],
                                     gw1[:], start=True, stop=False)
                    nc.tensor.matmul(lp32[0:nj, 0:128], ones1[0:1, 0:nj],
                                     b1row[0:1, :], start=False, stop=True)
                    nc.scalar.activation(ghrm8[0:nj, jt, :], lp32[0:nj, 0:128],
                                         AF.Prelu, alpha=a1)

            gmlp_t(h0, 0)

            nc.scalar.activation(pad0[:, 1:H + 1, 1:W + 1],
                                 h0[:, 0:HW].rearrange("p (h w) -> p h w", h=H), AF.Copy)
            taps = [(a, b) for a in range(3) for b in range(3)]
            for ri, (r0, nr) in enumerate(RCHUNKS):
                cpe = ps.tile([C, 512], F32, tag="sm", bufs=2, name=f"cpe_{ri}")
                for ti, (dy, dx) in enumerate(taps):
                    idx = (dy * 3 + dx) * 2
                    nc.tensor.matmul(cpe[:, 0:nr * W], cw[:, idx, :],
                                     pad0[:, r0 + dy:r0 + dy + nr, dx:dx + W],
                                     start=(ti == 0), stop=(ti == 8))
                nc.scalar.activation(convacc[:, r0 * W:(r0 + nr) * W],
                                     cpe[:, 0:nr * W], AF.Identity, bias=bia[:, 3:4])

            # ---------------- proj flatten + U/V staging ---------------------
            flatten_half(0)
            # x channel DMA overlaps the second median half
            nc.sync.dma_start(V[0:1, 0:HW], projn_d[0:HW])
            sq3 = work.tile([3, 2720], F16, tag="sq3", bufs=1, name="sq3")

            nc.sync.dma_start(U[0:1, 0:HW], projn_d[0:HW])
            flatten_half(1)
            for ch in (1, 2):
                nc.sync.dma_start(V[ch:ch + 1, 0:HW], projn_d[ch * HW:(ch + 1) * HW])
                nc.sync.dma_start(U[ch:ch + 1, 0:HW], projn_d[ch * HW:(ch + 1) * HW])
            # e'' = q.p - |p_j|^2/2 (same order as e' = 2q.p - |p_j|^2)
            nc.vector.tensor_tensor(sq3[0:3, 0:HW], V[0:3, 0:HW], V[0:3, 0:HW],
                                    ALU.mult)
            sqp = ps.tile([C, 1536], F32, tag="big3", bufs=2, name="sqp")
            for c0, ncn in CHUNKS6[:3]:
                nc.tensor.matmul(sqp[0:1, c0:c0 + ncn], ones3[:],
                                 sq3[:, c0:c0 + ncn], start=True, stop=True)
            hirow = work.tile([1, 2816], F16, tag="row", bufs=1, name="hirow")
            nc.scalar.activation(hirow[0:1, 0:1536], sqp[0:1, 0:1536],
                                 AF.Copy, scale=-0.5)
            nc.sync.dma_start(V[3:4, 0:1536], hirow[0:1, 0:1536])
            sqp2 = ps.tile([C, 1536], F32, tag="big3", bufs=2, name="sqp2")
            for c0, ncn in CHUNKS6[3:]:
                nc.tensor.matmul(sqp2[0:1, c0 - 1536:c0 - 1536 + ncn],
                                 ones3[:], sq3[:, c0:c0 + ncn], start=True, stop=True)
            nc.scalar.activation(hirow[0:1, 1536:HW], sqp2[0:1, 0:HW - 1536],
                                 AF.Copy, scale=-0.5)
            nc.sync.dma_start(V[3:4, 1536:HW], hirow[0:1, 1536:HW])

            # ---------------- p1: per-row te + sign, software-pipelined ------
            efs = {}

            def stage_ef(jt):
                i0, ni = PTILES[jt]
                ef = work.tile([C, 2720], F16, tag="ef", bufs=4, name=f"ef_{jt}")
                efs[jt] = ef
                for half, o0, on in ((0, 0, 1536), (1, 1536, HW - 1536)):
                    pp = ps.tile([C, 1536], F32, tag="big3", bufs=2,
                                 name=f"pp_{jt}_{half}")
                    for c0, ncn in (CHUNKS6[:3] if half == 0 else CHUNKS6[3:]):
                        nc.tensor.matmul(pp[0:ni, c0 - o0:c0 - o0 + ncn],
                                         U[0:4, i0:i0 + ni], V[0:4, c0:c0 + ncn],
                                         start=True, stop=True)
                    nc.scalar.activation(ef[0:ni, o0:o0 + on], pp[0:ni, 0:on],
                                         AF.Copy)

            msks = {}

            def p1_scan_a(jt):
                """top-8 + removal mask; the masked-row add runs on gpsimd."""
                i0, ni = PTILES[jt]
                ef = efs[jt]
                t8a = work.tile([C, 8], F16, tag="t8", bufs=6, name=f"t8a_{jt}")
                nc.vector.max(t8a[0:ni], ef[0:ni, 0:HW])
                v8f = work.tile([C, 1], F32, tag="v8f", bufs=12, name=f"v8f_{jt}")
                nc.vector.tensor_copy(v8f[0:ni], t8a[0:ni, 7:8])
                msk = work.tile([C, 2720], F16, tag="msk", bufs=3, name=f"msk_{jt}")
                msks[jt] = msk
                nc.vector.tensor_scalar(msk[0:ni, 0:HW], ef[0:ni, 0:HW],
                                        v8f[0:ni], NEG_F16,
                                        op0=ALU.is_ge, op1=ALU.mult)
                nc.gpsimd.tensor_tensor(msk[0:ni, 0:HW], ef[0:ni, 0:HW],
                                        msk[0:ni, 0:HW], ALU.add)

            def p1_scan_b(jt):
                i0, ni = PTILES[jt]
                t8b = work.tile([C, 8], F16, tag="t8", bufs=6, name=f"t8b_{jt}")
                nc.vector.max(t8b[0:ni], msks[jt][0:ni, 0:HW])
                # bias = -te + |te|*2^-11 + 4e-7
                tp1 = work.tile([C, 1], F32, tag="v8f", bufs=12, name=f"tp1_{jt}")
                nc.vector.tensor_scalar(tp1[0:ni], t8b[0:ni, 7:8], 2.0 ** -11, 0.0,
                                        op0=ALU.mult, op1=ALU.add)
                tab = work.tile([C, 1], F32, tag="v8f", bufs=12, name=f"tab_{jt}")
                nc.vector.scalar_tensor_tensor(tab[0:ni], t8b[0:ni, 7:8],
                                               -(2.0 ** -11), tp1[0:ni],
                                               ALU.mult, ALU.max)
                bv = work.tile([C, 1], F32, tag="v8f", bufs=12, name=f"bv_{jt}")
                nc.vector.scalar_tensor_tensor(bv[0:ni], tab[0:ni], 4.0e-7,
                                               t8b[0:ni, 7:8], ALU.add, ALU.subtract)
                nc.scalar.activation(Sbig[0:ni, jt, 0:HW], efs[jt][0:ni, 0:HW],
                                     AF.Sign, bias=bv[0:ni])

            stage_ef(0)
            stage_ef(1)
            p1_scan_a(0)
            stage_ef(2)
            p1_scan_a(1)
            for jt in range(22):
                if jt + 3 < 22:
                    stage_ef(jt + 3)
                if jt + 2 < 22:
                    p1_scan_a(jt + 2)
                p1_scan_b(jt)

            # ---------------- dense tail: agg1+q1, gmlp2, agg2+q2, conv ------
            DR = mybir.MatmulPerfMode.DoubleRow

            def agg_chunk(tgt, c0, ncn, first):
                # pairs of full 128-row tiles via fp8 DoubleRow (2 k-tiles per
                # matmul), then tiles 20 (128 rows) and 21 (12 rows) normally
                for pr in range(10):
                    nc.tensor.matmul(tgt,
                                     ghrm8[:, 2 * pr:2 * pr + 2, :],
                                     Sbig[:, 2 * pr:2 * pr + 2, c0:c0 + ncn],
                                     start=(pr == 0), stop=False,
                                     perf_mode=DR)
                for jt in (20, 21):
                    j0, nj = PTILES[jt]
                    nc.tensor.matmul(tgt,
                                     ghrm8[0:nj, jt, :],
                                     Sbig[0:nj, jt, c0:c0 + ncn],
                                     start=False, stop=(jt == 21))

            def q_half(qp, h_in, mts, half, o0):
                for c0, ncn in (CHUNKS6[:3] if half == 0 else CHUNKS6[3:]):
                    nc.tensor.matmul(qp[:, c0 - o0:c0 - o0 + ncn], qw1[:],
                                     h_in[:, c0:c0 + ncn], start=True, stop=False)
                    nc.tensor.matmul(qp[:, c0 - o0:c0 - o0 + ncn], qw2[:],
                                     mts[:, c0 - o0:c0 - o0 + ncn],
                                     start=False, stop=True)

            def agg_q(it, h_in, pad=None):
                """Aggregation (G-chunk first) fused with the q update so the
                q matmuls overlap the remaining aggregation chunks."""
                A = ps.tile([C, 1536], F32, tag="big3", bufs=2, name=f"agg{it}A")
                B = ps.tile([C, 1536], F32, tag="big3", bufs=2, name=f"agg{it}B")
                agg_chunk(B[:, 1024:1165], 2560, 141, True)
                gcol = sb.tile([C, 1], F32, tag=f"gcol_{it}")
                nc.scalar.activation(gcol[:], B[:, 1164:1165], AF.Copy)
                for c0 in (0, 512, 1024):
                    agg_chunk(A[:, c0:c0 + 512], c0, 512, False)
                mtsA = work.tile([C, 1536], F16, tag="mtsA", bufs=1,
                                 name=f"mtsA_{it}")
                nc.scalar.activation(mtsA[:], A[:, 0:1536], AF.Identity,
                                     bias=gcol[:])
                agg_chunk(B[:, 0:512], 1536, 512, False)
                h_out = work.tile([C, 2720], F16, tag="h", bufs=2, name=f"h_{it}")
                qpA = ps.tile([C, 1536], F32, tag="big3", bufs=2,
                              name=f"qp_{it}_0")
                q_half(qpA, h_in, mtsA, 0, 0)
                nc.scalar.activation(h_out[:, 0:1536], qpA[:, 0:1536], AF.Prelu,
                                     bias=bia[:, 2:3], alpha=qa)
                if pad is not None:
                    nc.scalar.activation(
                        pad[:, 1:26, 1:W + 1],
                        h_out[:, 0:1500].rearrange("p (h w) -> p h w", w=W),
                        AF.Copy)
                agg_chunk(B[:, 512:1024], 2048, 512, False)
                mtsB = work.tile([C, 1536], F16, tag="mtsB", bufs=1,
                                 name=f"mtsB_{it}")
                nc.scalar.activation(mtsB[:, 0:1164], B[:, 0:1164], AF.Identity,
                                     bias=gcol[:])
                qpB = ps.tile([C, 1536], F32, tag="big3", bufs=2,
                              name=f"qp_{it}_1")
                q_half(qpB, h_in, mtsB, 1, 1536)
                nc.scalar.activation(h_out[:, 1536:HW], qpB[:, 0:HW - 1536],
                                     AF.Prelu, bias=bia[:, 2:3], alpha=qa)
                if pad is not None:
                    nc.scalar.activation(
                        pad[:, 26:H + 1, 1:W + 1],
                        h_out[:, 1500:HW].rearrange("p (h w) -> p h w", w=W),
                        AF.Copy)
                return h_out

            h1 = agg_q(0, h0)
            gmlp_t(h1, 1)
            h2 = agg_q(1, h1, pad=pad1)

            oc = work.tile([C, 2720], F32, tag="bigf32", bufs=1, name="oc")
            for ri, (r0, nr) in enumerate(RCHUNKS):
                cpe = ps.tile([C, 512], F32, tag="sm", bufs=2, name=f"cp2_{ri}")
                for ti, (dy, dx) in enumerate(taps):
                    idx = (dy * 3 + dx) * 2 + 1
                    nc.tensor.matmul(cpe[:, 0:nr * W], cw[:, idx, :],
                                     pad1[:, r0 + dy:r0 + dy + nr, dx:dx + W],
                                     start=(ti == 0), stop=(ti == 8))
                nc.vector.tensor_tensor(oc[:, r0 * W:(r0 + nr) * W],
                                        cpe[:, 0:nr * W],
                                        convacc[:, r0 * W:(r0 + nr) * W], ALU.add)
                if ri == 2:
                    nc.sync.dma_start(out_d[:, 0:1440], oc[:, 0:1440])
                elif ri == 5:
                    nc.sync.dma_start(out_d[:, 1440:2700], oc[:, 1440:2700])

    nc.compile()
    return nc


def kernel(cnn_encoder_output, original_input, xy,
           g_w0, g_b0, g_a0, g_w1, g_b1, g_a1,
           q_w, q_b, q_a, conv_w, conv_b,
           gnn_iterations, k, use_half_precision, _trace=False):
    assert int(gnn_iterations) == 2 and int(k) == 16 and int(use_half_precision) == 0

    cnn = np.asarray(cnn_encoder_output, dtype=np.float32)
    orig = np.asarray(original_input, dtype=np.float32)
    xy = np.asarray(xy, dtype=np.float32)
    a0, a1, qa = float(np.ravel(g_a0)[0]), float(np.ravel(g_a1)[0]), float(np.ravel(q_a)[0])

    key = (a0, a1, qa)
    if key not in _cache:
        _cache[key] = _build(a0, a1, qa)
    nc = _cache[key]

    g_w0 = np.asarray(g_w0, np.float32)
    g_w1 = np.asarray(g_w1, np.float32)
    q_w = np.asarray(q_w, np.float32)
    conv_w = np.asarray(conv_w, np.float32)

    gw0T = np.ascontiguousarray(g_w0.T).astype(np.float16)
    gw1T = np.ascontiguousarray(g_w1.T).astype(np.float16)
    qw1T = np.ascontiguousarray(q_w[:, :C].T).astype(np.float16)
    qw2T = np.ascontiguousarray(q_w[:, C:].T / float(2 * K)).astype(np.float16)
    cwT = np.empty((C, 18, C), np.float16)
    for dy in range(3):
        for dx in range(3):
            for kh in range(2):
                idx = (dy * 3 + dx) * 2 + kh
                cwT[:, idx, :] = conv_w[:, kh * C:(kh + 1) * C, dy, dx].T.astype(np.float16)
    biases = np.stack([np.asarray(g_b0, np.float32), np.asarray(g_b1, np.float32),
                       np.asarray(q_b, np.float32), np.asarray(conv_b, np.float32)],
                      axis=1)
    b1row = np.stack([np.asarray(g_b1, np.float16),
                      np.asarray(g_b0, np.float16)], axis=0)
    ident = np.eye(C, dtype=np.float16)
    uvc = np.zeros((2, 8, 2816), np.float16)
    uvc[0, 3] = 1.0

    shared = dict(gw0T=gw0T, gw1T=gw1T, qw1T=qw1T, qw2T=qw2T, convwT=cwT,
                  biases=np.ascontiguousarray(biases),
                  b1row=np.ascontiguousarray(b1row), ident=ident, uvc=uvc)
    in_maps = []
    for n in range(N):
        chans = np.stack([xy[n, 0], xy[n, 1], orig[n, 3]], axis=0)      # [3, 360, 480]
        blocks = chans.reshape(3, H, 8, W, 8).transpose(0, 1, 3, 2, 4).reshape(3 * HW, 64)
        blocks = (-blocks).astype(np.float16)
        pad = np.zeros((8192, 64), np.float16)
        pad[:3 * HW] = blocks
        psrcb = pad.reshape(16, 4, 128, 64).transpose(0, 2, 1, 3)
        in_maps.append(dict(h0=np.ascontiguousarray(
                                cnn[n].reshape(C, HW).astype(np.float16)),
                            psrcb=np.ascontiguousarray(psrcb), **shared))

    if _trace:
        _ensure_ntff_hook()
    res = run_bass_kernel_spmd(nc, in_maps, core_ids=list(range(N)), trace=_trace,
                               trace_cores=list(range(N)) if _trace else None)
    out = np.stack([res.results[n]["out"].reshape(C, H, W).astype(np.float32)
                    for n in range(N)])
    if _trace:
        kernel._last_results = res
    return out


# revision 35
# speedup vs baseline: 1.0257x; 1.0257x over previous
"""Trainium2 Bass kernel for EnetGnn (gnn_message_passing).

Data-parallel over batch N=8, one sample per NeuronCore. Per-core design:

1. Median pool: host stages negated fp16 blocks in [16, 128, 4, 64] tiles so
   each load is one contiguous 64KB DMA. DVE max8/match_replace rank-32
   rounds; medians flattened via two half PE transposes + DMA so the x
   channel stages while the second half of the median still runs.
2. KNN threshold: e'[i,j] = 2p_i.p_j - |p_j|^2 via K=4 fp16 matmuls into
   double-buffered 3-bank psum halves, ACT-evacuated to fp16 ef. Per-row
   16th-largest te via max8 + is_ge mask removal + max8 (all DVE).
3. S = Sign(ef - te + eps) on the ACT engine with per-row bias (no phase-2
   matmul recompute). S tiles land in one [128, 22, 2720] fp8 SBUF tensor
   with a ones-column for G.
4. All heavy matmul work (aggregation iters 1+2, q updates, g-MLP 2, conv)
   runs as one dense tail stream to keep the PE at its hot clock. The
   ones-column gives G; a rank-1 matmul broadcasts G so mts = G + S@gh
   evacuates at natural scale to fp16, making the q update all-fp16.
"""
import numpy as np
import concourse.bass as bass
import concourse.bacc as bacc
import concourse.mybir as mybir
import concourse.tile as tile
from concourse.bass_utils import run_bass_kernel_spmd

F32 = mybir.dt.float32
F16 = mybir.dt.float16
F8 = mybir.dt.float8e4
AF = mybir.ActivationFunctionType
ALU = mybir.AluOpType

N, C, H, W = 8, 128, 45, 60
HW = H * W                      # 2700
K = 16
NEG_F16 = -60000.0

CHUNKS6 = [(0, 512), (512, 512), (1024, 512), (1536, 512), (2048, 512), (2560, 140)]
AGG_CHUNKS = [(0, 512), (512, 512), (1024, 512), (1536, 512), (2048, 512), (2560, 141)]
PTILES = [(t * 128, 128) for t in range(21)] + [(2688, 12)]
RCHUNKS = [(0, 8), (8, 8), (16, 8), (24, 8), (32, 8), (40, 5)]
TGROUPS = [list(range(0, 8)), list(range(8, 16)), list(range(16, 22))]

_cache = {}


def _ensure_ntff_hook():
    import sys
    import types
    try:
        from antenv.axon_hooks import get_axon_ntff_profile_hook  # noqa: F401
        return
    except ImportError:
        pass
    try:
        mod = types.ModuleType("antenv.axon_hooks")
        mod._hook = None

        def set_axon_ntff_profile_hook(h):
            mod._hook = h

        def get_axon_ntff_profile_hook():
            return mod._hook

        mod.set_axon_ntff_profile_hook = set_axon_ntff_profile_hook
        mod.get_axon_ntff_profile_hook = get_axon_ntff_profile_hook
        sys.modules["antenv.axon_hooks"] = mod
        import antenv
        antenv.axon_hooks = mod
        from trn_agent_boot.trn_boot import _ntff_profile_via_ctypes
        hook = _ntff_profile_via_ctypes("/opt/axon/libaxon_pjrt.so")
        if hook is not None:
            mod.set_axon_ntff_profile_hook(hook)
    except Exception as e:  # profiling is best-effort
        print(f"ntff hook injection failed: {e}")


def _build(a0, a1, qa):
    nc = bacc.Bacc("TRN2", target_bir_lowering=False, debug=False, num_devices=8)

    h0_d = nc.dram_tensor("h0", (C, HW), F16, kind="ExternalInput")
    psrcb_d = nc.dram_tensor("psrcb", (16, 128, 4, 64), F16, kind="ExternalInput")
    gw0_d = nc.dram_tensor("gw0T", (C, C), F16, kind="ExternalInput")
    gw1_d = nc.dram_tensor("gw1T", (C, C), F16, kind="ExternalInput")
    qw1_d = nc.dram_tensor("qw1T", (C, C), F16, kind="ExternalInput")
    qw2_d = nc.dram_tensor("qw2T", (C, C), F16, kind="ExternalInput")
    cw_d = nc.dram_tensor("convwT", (C, 18, C), F16, kind="ExternalInput")
    bias_d = nc.dram_tensor("biases", (C, 4), F32, kind="ExternalInput")
    b1row_d = nc.dram_tensor("b1row", (2, C), F16, kind="ExternalInput")
    ident_d = nc.dram_tensor("ident", (C, C), F16, kind="ExternalInput")
    uvc_d = nc.dram_tensor("uvc", (2, 8, 2816), F16, kind="ExternalInput")
    out_d = nc.dram_tensor("out", (C, HW), F32, kind="ExternalOutput")

    with tile.TileContext(nc) as tc:
        with tc.tile_pool(name="sb", bufs=1) as sb, \
             tc.tile_pool(name="work", bufs=2) as work, \
             tc.tile_pool(name="ps", bufs=1, space="PSUM") as ps, \
             tc.tile_pool(name="dram", bufs=1, space="DRAM") as dram:

            projn_d = dram.tile([8192], F16, tag="projn_d")

            # median block DMAs first so the DVE phase starts immediately
            blks = []
            for g in range(16):
                blk = work.tile([128, 4, 64], F16, tag="blk", bufs=8,
                                name=f"blk_{g}")
                nc.sync.dma_start(blk[:], psrcb_d[g])
                blks.append(blk)

            # ---------------- persistent SBUF ----------------
            h0 = sb.tile([C, 2720], F16, tag="h0")
            nc.sync.dma_start(h0[:, 0:HW], h0_d[:])
            gw0 = sb.tile([C, C], F16, tag="gw0")
            nc.sync.dma_start(gw0[:], gw0_d[:])
            gw1 = sb.tile([C, C], F16, tag="gw1")
            nc.sync.dma_start(gw1[:], gw1_d[:])
            qw1 = sb.tile([C, C], F16, tag="qw1")
            nc.sync.dma_start(qw1[:], qw1_d[:])
            qw2 = sb.tile([C, C], F16, tag="qw2")
            nc.sync.dma_start(qw2[:], qw2_d[:])
            cw = sb.tile([C, 18, C], F16, tag="cw")
            nc.sync.dma_start(cw[:], cw_d[:])
            bia = sb.tile([C, 4], F32, tag="bias")
            nc.sync.dma_start(bia[:], bias_d[:])
            b1row = sb.tile([2, C], F16, tag="b1row")
            nc.sync.dma_start(b1row[:], b1row_d[:])
            ones1 = sb.tile([1, C], F16, tag="ones1")
            ident = sb.tile([C, C], F16, tag="ident")
            nc.sync.dma_start(ident[:], ident_d[:])

            U = sb.tile([8, 2816], F16, tag="U")       # [q; 1]
            nc.sync.dma_start(U[:], uvc_d[0])
            V = sb.tile([8, 2816], F16, tag="V")       # [q; -|p|^2/2]
            nc.sync.dma_start(V[:], uvc_d[1])
            Sbig = sb.tile([C, 22, 2720], F8, tag="Sbig")
            ghrm8 = sb.tile([C, 22, 128], F8, tag="ghrm8")
            M8 = sb.tile([C, 64, 8], F16, tag="M8")
            Mt = sb.tile([64, C], F16, tag="Mt")
            pad0 = sb.tile([C, H + 2, W + 2], F16, tag="pad0")
            pad1 = sb.tile([C, H + 2, W + 2], F16, tag="pad1")
            convacc = sb.tile([C, 2720], F32, tag="convacc")
            ones3 = sb.tile([3, 1], F16, tag="ones3")

            # memsets on gpsimd (DVE stays on the median path)
            nc.gpsimd.memset(Sbig[:, :, HW:HW + 1], 1.0)   # ones-cols for G
            nc.gpsimd.memset(pad0[:], 0.0)
            nc.gpsimd.memset(pad1[:], 0.0)
            nc.gpsimd.memset(ones3[:], 1.0)
            nc.gpsimd.memset(ones1[:], 1.0)

            # ---------------- median pooling + split flatten -----------------
            def median_range(glo, ghi):
                for g in range(glo, ghi):
                    blk = blks[g]
                    for s in range(4):
                        mm8 = work.tile([128, 8], F16, tag="mm8", bufs=8)
                        for rnd in range(3):
                            nc.vector.max(mm8[:], blk[:, s, :])
                            nc.vector.match_replace(blk[:, s, :], mm8[:],
                                                    blk[:, s, :], NEG_F16)
                        nc.vector.max(M8[:, g * 4 + s, :], blk[:, s, :])

            def flatten_half(half):
                lo, nc_ = (0, 32) if half == 0 else (32, 32)
                mtp = ps.tile([C, 1024], F16, tag="sm", bufs=2, name=f"mtp{half}")
                Mcols = M8[:, lo:lo + 32, 7:8].rearrange("p a b -> p (a b)")
                nc.tensor.transpose(mtp[0:32, 0:128], Mcols, ident[:])
                nc.scalar.activation(Mt[lo:lo + 32, :], mtp[0:32, 0:128], AF.Copy)
                projn_r = projn_d.rearrange("(a b) -> a b", b=128)
                nc.sync.dma_start(projn_r[lo:lo + 32, :], Mt[lo:lo + 32, :])

            median_range(0, 8)
            median_range(8, 16)

            # ---------------- iter-1 g-MLP + conv h0-half (under median) -----
            def mlp_layer(w, h_in, out, it, lab, bias, alpha):
                for half, o0, on in ((0, 0, 1536), (1, 1536, HW - 1536)):
                    gp = ps.tile([C, 1536], F32, tag="big3", bufs=2,
                                 name=f"{lab}_{it}_{half}")
                    for c0, ncn in (CHUNKS6[:3] if half == 0 else CHUNKS6[3:]):
                        nc.tensor.matmul(gp[:, c0 - o0:c0 - o0 + ncn], w[:],
                                         h_in[:, c0:c0 + ncn], start=True, stop=True)
                    nc.scalar.activation(out[:, o0:o0 + on], gp[:, 0:on], AF.Prelu,
                                         bias=bias, alpha=alpha)

            def gmlp_t(h_in, it):
                """g-MLP: layer 1 straight, layer 2 in transposed orientation
                writing gh2^T tiles directly into ghrm8 (fp8). The b1 bias is
                added via a rank-1 matmul (per-feature = free dim here)."""
                gh1 = work.tile([C, 2720], F16, tag="gh", bufs=1, name=f"gh1_{it}")
                mlp_layer(gw0, h_in, gh1, it, "g1", bia[:, 0:1], a0)
                for jt, (j0, nj) in enumerate(PTILES):
                    lp32 = ps.tile([C, 512], F32, tag="sm", bufs=2,
                                   name=f"l2t_{it}_{jt}")
                    nc.tensor.matmul(lp32[0:nj, 0:128], gh1[:, j0:j0 + nj],
                                     gw1[:], start=True, stop=False)
                    nc.tensor.matmul(lp32[0:nj, 0:128], ones1[0:1, 0:nj],
                                     b1row[0:1, :], start=False, stop=True)
                    nc.scalar.activation(ghrm8[0:nj, jt, :], lp32[0:nj, 0:128],
                                         AF.Prelu, alpha=a1)

            gmlp_t(h0, 0)

            nc.scalar.activation(pad0[:, 1:H + 1, 1:W + 1],
                                 h0[:, 0:HW].rearrange("p (h w) -> p h w", h=H), AF.Copy)
            taps = [(a, b) for a in range(3) for b in range(3)]
            for ri, (r0, nr) in enumerate(RCHUNKS):
                cpe = ps.tile([C, 512], F32, tag="sm", bufs=2, name=f"cpe_{ri}")
                for ti, (dy, dx) in enumerate(taps):
                    idx = (dy * 3 + dx) * 2
                    nc.tensor.matmul(cpe[:, 0:nr * W], cw[:, idx, :],
                                     pad0[:, r0 + dy:r0 + dy + nr, dx:dx + W],
                                     start=(ti == 0), stop=(ti == 8))
                nc.scalar.activation(convacc[:, r0 * W:(r0 + nr) * W],
                                     cpe[:, 0:nr * W], AF.Identity, bias=bia[:, 3:4])

            # ---------------- proj flatten + U/V staging ---------------------
            flatten_half(0)
            # x channel DMA overlaps the second median half
            nc.sync.dma_start(V[0:1, 0:HW], projn_d[0:HW])
            sq3 = work.tile([3, 2720], F16, tag="sq3", bufs=1, name="sq3")

            nc.sync.dma_start(U[0:1, 0:HW], projn_d[0:HW])
            flatten_half(1)
            for ch in (1, 2):
                nc.sync.dma_start(V[ch:ch + 1, 0:HW], projn_d[ch * HW:(ch + 1) * HW])
                nc.sync.dma_start(U[ch:ch + 1, 0:HW], projn_d[ch * HW:(ch + 1) * HW])
            # e'' = q.p - |p_j|^2/2 (same order as e' = 2q.p - |p_j|^2)
            nc.vector.tensor_tensor(sq3[0:3, 0:HW], V[0:3, 0:HW], V[0:3, 0:HW],
                                    ALU.mult)
            sqp = ps.tile([C, 1536], F32, tag="big3", bufs=2, name="sqp")
            for c0, ncn in CHUNKS6[:3]:
                nc.tensor.matmul(sqp[0:1, c0:c0 + ncn], ones3[:],
                                 sq3[:, c0:c0 + ncn], start=True, stop=True)
            hirow = work.tile([1, 2816], F16, tag="row", bufs=1, name="hirow")
            nc.scalar.activation(hirow[0:1, 0:1536], sqp[0:1, 0:1536],
                                 AF.Copy, scale=-0.5)
            nc.sync.dma_start(V[3:4, 0:1536], hirow[0:1, 0:1536])
            sqp2 = ps.tile([C, 1536], F32, tag="big3", bufs=2, name="sqp2")
            for c0, ncn in CHUNKS6[3:]:
                nc.tensor.matmul(sqp2[0:1, c0 - 1536:c0 - 1536 + ncn],
                                 ones3[:], sq3[:, c0:c0 + ncn], start=True, stop=True)
            nc.scalar.activation(hirow[0:1, 1536:HW], sqp2[0:1, 0:HW - 1536],
                                 AF.Copy, scale=-0.5)
            nc.sync.dma_start(V[3:4, 1536:HW], hirow[0:1, 1536:HW])

            # ---------------- p1: per-row te + sign, software-pipelined ------
            efs = {}

            def stage_ef(jt):
                i0, ni = PTILES[jt]
                ef = work.tile([C, 2720], F16, tag="ef", bufs=4, name=f"ef_{jt}")
                efs[jt] = ef
                for half, o0, on in ((0, 0, 1536), (1, 1536, HW - 1536)):
                    pp = ps.tile([C, 1536], F32, tag="big3", bufs=2,
                                 name=f"pp_{jt}_{half}")
                    for c0, ncn in (CHUNKS6[:3] if half == 0 else CHUNKS6[3:]):
                        nc.tensor.matmul(pp[0:ni, c0 - o0:c0 - o0 + ncn],
                                         U[0:4, i0:i0 + ni], V[0:4, c0:c0 + ncn],
                                         start=True, stop=True)
                    nc.scalar.activation(ef[0:ni, o0:o0 + on], pp[0:ni, 0:on],
                                         AF.Copy)

            msks = {}

            def p1_scan_a(jt):
                """top-8 + removal mask; the masked-row add runs on gpsimd."""
                i0, ni = PTILES[jt]
                ef = efs[jt]
                t8a = work.tile([C, 8], F16, tag="t8", bufs=6, name=f"t8a_{jt}")
                nc.vector.max(t8a[0:ni], ef[0:ni, 0:HW])
                v8f = work.tile([C, 1], F32, tag="v8f", bufs=12, name=f"v8f_{jt}")
                nc.vector.tensor_copy(v8f[0:ni], t8a[0:ni, 7:8])
                msk = work.tile([C, 2720], F16, tag="msk", bufs=3, name=f"msk_{jt}")
                msks[jt] = msk
                nc.vector.tensor_scalar(msk[0:ni, 0:HW], ef[0:ni, 0:HW],
                                        v8f[0:ni], NEG_F16,
                                        op0=ALU.is_ge, op1=ALU.mult)
                nc.vector.tensor_tensor(msk[0:ni, 0:HW], ef[0:ni, 0:HW],
                                        msk[0:ni, 0:HW], ALU.add)

            def p1_scan_b(jt):
                i0, ni = PTILES[jt]
                t8b = work.tile([C, 8], F16, tag="t8", bufs=6, name=f"t8b_{jt}")
                nc.vector.max(t8b[0:ni], msks[jt][0:ni, 0:HW])
                # bias = -te + |te|*2^-11 + 4e-7
                tp1 = work.tile([C, 1], F32, tag="v8f", bufs=12, name=f"tp1_{jt}")
                nc.vector.tensor_scalar(tp1[0:ni], t8b[0:ni, 7:8], 2.0 ** -11, 0.0,
                                        op0=ALU.mult, op1=ALU.add)
                tab = work.tile([C, 1], F32, tag="v8f", bufs=12, name=f"tab_{jt}")
                nc.vector.scalar_tensor_tensor(tab[0:ni], t8b[0:ni, 7:8],
                                               -(2.0 ** -11), tp1[0:ni],
                                               ALU.mult, ALU.max)
                bv = work.tile([C, 1], F32, tag="v8f", bufs=12, name=f"bv_{jt}")
                nc.vector.scalar_tensor_tensor(bv[0:ni], tab[0:ni], 4.0e-7,
                                               t8b[0:ni, 7:8], ALU.add, ALU.subtract)
                nc.scalar.activation(Sbig[0:ni, jt, 0:HW], efs[jt][0:ni, 0:HW],
                                     AF.Sign, bias=bv[0:ni])

            stage_ef(0)
            stage_ef(1)
            p1_scan_a(0)
            stage_ef(2)
            p1_scan_a(1)
            for jt in range(22):
                if jt + 3 < 22:
                    stage_ef(jt + 3)
                if jt + 2 < 22:
                    p1_scan_a(jt + 2)
                p1_scan_b(jt)

            # ---------------- dense tail: agg1+q1, gmlp2, agg2+q2, conv ------
            DR = mybir.MatmulPerfMode.DoubleRow

            def agg_chunk(tgt, c0, ncn, first):
                # pairs of full 128-row tiles via fp8 DoubleRow (2 k-tiles per
                # matmul), then tiles 20 (128 rows) and 21 (12 rows) normally
                for pr in range(10):
                    nc.tensor.matmul(tgt,
                                     ghrm8[:, 2 * pr:2 * pr + 2, :],
                                     Sbig[:, 2 * pr:2 * pr + 2, c0:c0 + ncn],
                                     start=(pr == 0), stop=False,
                                     perf_mode=DR)
                for jt in (20, 21):
                    j0, nj = PTILES[jt]
                    nc.tensor.matmul(tgt,
                                     ghrm8[0:nj, jt, :],
                                     Sbig[0:nj, jt, c0:c0 + ncn],
                                     start=False, stop=(jt == 21))

            def q_half(qp, h_in, mts, half, o0):
                for c0, ncn in (CHUNKS6[:3] if half == 0 else CHUNKS6[3:]):
                    nc.tensor.matmul(qp[:, c0 - o0:c0 - o0 + ncn], qw1[:],
                                     h_in[:, c0:c0 + ncn], start=True, stop=False)
                    nc.tensor.matmul(qp[:, c0 - o0:c0 - o0 + ncn], qw2[:],
                                     mts[:, c0 - o0:c0 - o0 + ncn],
                                     start=False, stop=True)

            def agg_q(it, h_in, pad=None):
                """Aggregation (G-chunk first) fused with the q update so the
                q matmuls overlap the remaining aggregation chunks."""
                A = ps.tile([C, 1536], F32, tag="big3", bufs=2, name=f"agg{it}A")
                B = ps.tile([C, 1536], F32, tag="big3", bufs=2, name=f"agg{it}B")
                agg_chunk(B[:, 1024:1165], 2560, 141, True)
                gcol = sb.tile([C, 1], F32, tag=f"gcol_{it}")
                nc.scalar.activation(gcol[:], B[:, 1164:1165], AF.Copy)
                for c0 in (0, 512, 1024):
                    agg_chunk(A[:, c0:c0 + 512], c0, 512, False)
                mtsA = work.tile([C, 1536], F16, tag="mtsA", bufs=1,
                                 name=f"mtsA_{it}")
                nc.scalar.activation(mtsA[:], A[:, 0:1536], AF.Identity,
                                     bias=gcol[:])
                agg_chunk(B[:, 0:512], 1536, 512, False)
                h_out = work.tile([C, 2720], F16, tag="h", bufs=2, name=f"h_{it}")
                qpA = ps.tile([C, 1536], F32, tag="big3", bufs=2,
                              name=f"qp_{it}_0")
                q_half(qpA, h_in, mtsA, 0, 0)
                nc.scalar.activation(h_out[:, 0:1536], qpA[:, 0:1536], AF.Prelu,
                                     bias=bia[:, 2:3], alpha=qa)
                if pad is not None:
                    nc.scalar.activation(
                        pad[:, 1:26, 1:W + 1],
                        h_out[:, 0:1500].rearrange("p (h w) -> p h w", w=W),
                        AF.Copy)
                agg_chunk(B[:, 512:1024], 2048, 512, False)
                mtsB = work.tile([C, 1536], F16, tag="mtsB", bufs=1,
                                 name=f"mtsB_{it}")
                nc.scalar.activation(mtsB[:, 0:1164], B[:, 0:1164], AF.Identity,
                                     bias=gcol[:])
                qpB = ps.tile([C, 1536], F32, tag="big3", bufs=2,
                              name=f"qp_{it}_1")
                q_half(qpB, h_in, mtsB, 1, 1536)
                nc.scalar.activation(h_out[:, 1536:HW], qpB[:, 0:HW - 1536],
                                     AF.Prelu, bias=bia[:, 2:3], alpha=qa)
                if pad is not None:
                    nc.scalar.activation(
                        pad[:, 26:H + 1, 1:W + 1],
                        h_out[:, 1500:HW].rearrange("p (h w) -> p h w", w=W),
                        AF.Copy)
                return h_out

            h1 = agg_q(0, h0)
            gmlp_t(h1, 1)
            h2 = agg_q(1, h1, pad=pad1)

            oc = work.tile([C, 2720], F32, tag="bigf32", bufs=1, name="oc")
            for ri, (r0, nr) in enumerate(RCHUNKS):
                cpe = ps.tile([C, 512], F32, tag="sm", bufs=2, name=f"cp2_{ri}")
                for ti, (dy, dx) in enumerate(taps):
                    idx = (dy * 3 + dx) * 2 + 1
                    nc.tensor.matmul(cpe[:, 0:nr * W], cw[:, idx, :],
                                     pad1[:, r0 + dy:r0 + dy + nr, dx:dx + W],
                                     start=(ti == 0), stop=(ti == 8))
                nc.vector.tensor_tensor(oc[:, r0 * W:(r0 + nr) * W],
                                        cpe[:, 0:nr * W],
                                        convacc[:, r0 * W:(r0 + nr) * W], ALU.add)
                if ri == 2:
                    nc.sync.dma_start(out_d[:, 0:1440], oc[:, 0:1440])
                elif ri == 5:
                    nc.sync.dma_start(out_d[:, 1440:2700], oc[:, 1440:2700])

    nc.compile()
    return nc


def kernel(cnn_encoder_output, original_input, xy,
           g_w0, g_b0, g_a0, g_w1, g_b1, g_a1,
           q_w, q_b, q_a, conv_w, conv_b,
           gnn_iterations, k, use_half_precision, _trace=False):
    assert int(gnn_iterations) == 2 and int(k) == 16 and int(use_half_precision) == 0

    cnn = np.asarray(cnn_encoder_output, dtype=np.float32)
    orig = np.asarray(original_input, dtype=np.float32)
    xy = np.asarray(xy, dtype=np.float32)
    a0, a1, qa = float(np.ravel(g_a0)[0]), float(np.ravel(g_a1)[0]), float(np.ravel(q_a)[0])

    key = (a0, a1, qa)
    if key not in _cache:
        _cache[key] = _build(a0, a1, qa)
    nc = _cache[key]

    g_w0 = np.asarray(g_w0, np.float32)
    g_w1 = np.asarray(g_w1, np.float32)
    q_w = np.asarray(q_w, np.float32)
    conv_w = np.asarray(conv_w, np.float32)

    gw0T = np.ascontiguousarray(g_w0.T).astype(np.float16)
    gw1T = np.ascontiguousarray(g_w1.T).astype(np.float16)
    qw1T = np.ascontiguousarray(q_w[:, :C].T).astype(np.float16)
    qw2T = np.ascontiguousarray(q_w[:, C:].T / float(2 * K)).astype(np.float16)
    cwT = np.empty((C, 18, C), np.float16)
    for dy in range(3):
        for dx in range(3):
            for kh in range(2):
                idx = (dy * 3 + dx) * 2 + kh
                cwT[:, idx, :] = conv_w[:, kh * C:(kh + 1) * C, dy, dx].T.astype(np.float16)
    biases = np.stack([np.asarray(g_b0, np.float32), np.asarray(g_b1, np.float32),
                       np.asarray(q_b, np.float32), np.asarray(conv_b, np.float32)],
                      axis=1)
    b1row = np.stack([np.asarray(g_b1, np.float16),
                      np.asarray(g_b0, np.float16)], axis=0)
    ident = np.eye(C, dtype=np.float16)
    uvc = np.zeros((2, 8, 2816), np.float16)
    uvc[0, 3] = 1.0

    shared = dict(gw0T=gw0T, gw1T=gw1T, qw1T=qw1T, qw2T=qw2T, convwT=cwT,
                  biases=np.ascontiguousarray(biases),
                  b1row=np.ascontiguousarray(b1row), ident=ident, uvc=uvc)
    in_maps = []
    for n in range(N):
        chans = np.stack([xy[n, 0], xy[n, 1], orig[n, 3]], axis=0)      # [3, 360, 480]
        blocks = chans.reshape(3, H, 8, W, 8).transpose(0, 1, 3, 2, 4).reshape(3 * HW, 64)
        blocks = (-blocks).astype(np.float16)
        pad = np.zeros((8192, 64), np.float16)
        pad[:3 * HW] = blocks
        psrcb = pad.reshape(16, 4, 128, 64).transpose(0, 2, 1, 3)
        in_maps.append(dict(h0=np.ascontiguousarray(
                                cnn[n].reshape(C, HW).astype(np.float16)),
                            psrcb=np.ascontiguousarray(psrcb), **shared))

    if _trace:
        _ensure_ntff_hook()
    res = run_bass_kernel_spmd(nc, in_maps, core_ids=list(range(N)), trace=_trace,
                               trace_cores=list(range(N)) if _trace else None)
    out = np.stack([res.results[n]["out"].reshape(C, H, W).astype(np.float32)
                    for n in range(N)])
    if _trace:
        kernel._last_results = res
    return out


# revision 36
# speedup vs baseline: 1.0435x; 1.0174x over previous
"""Trainium2 Bass kernel for EnetGnn (gnn_message_passing).

Data-parallel over batch N=8, one sample per NeuronCore. Per-core design:

1. Median pool: host stages negated fp16 blocks in [16, 128, 4, 64] tiles so
   each load is one contiguous 64KB DMA. DVE max8/match_replace rank-32
   rounds; medians flattened via two half PE transposes + DMA so the x
   channel stages while the second half of the median still runs.
2. KNN threshold: e'[i,j] = 2p_i.p_j - |p_j|^2 via K=4 fp16 matmuls into
   double-buffered 3-bank psum halves, ACT-evacuated to fp16 ef. Per-row
   16th-largest te via max8 + is_ge mask removal + max8 (all DVE).
3. S = Sign(ef - te + eps) on the ACT engine with per-row bias (no phase-2
   matmul recompute). S tiles land in one [128, 22, 2720] fp8 SBUF tensor
   with a ones-column for G.
4. All heavy matmul work (aggregation iters 1+2, q updates, g-MLP 2, conv)
   runs as one dense tail stream to keep the PE at its hot clock. The
   ones-column gives G; a rank-1 matmul broadcasts G so mts = G + S@gh
   evacuates at natural scale to fp16, making the q update all-fp16.
"""
import numpy as np
import concourse.bass as bass
import concourse.bacc as bacc
import concourse.mybir as mybir
import concourse.tile as tile
from concourse.bass_utils import run_bass_kernel_spmd

F32 = mybir.dt.float32
F16 = mybir.dt.float16
F8 = mybir.dt.float8e4
AF = mybir.ActivationFunctionType
ALU = mybir.AluOpType

N, C, H, W = 8, 128, 45, 60
HW = H * W                      # 2700
K = 16
NEG_F16 = -60000.0

CHUNKS6 = [(0, 512), (512, 512), (1024, 512), (1536, 512), (2048, 512), (2560, 140)]
AGG_CHUNKS = [(0, 512), (512, 512), (1024, 512), (1536, 512), (2048, 512), (2560, 141)]
PTILES = [(t * 128, 128) for t in range(21)] + [(2688, 12)]
RCHUNKS = [(0, 8), (8, 8), (16, 8), (24, 8), (32, 8), (40, 5)]
TGROUPS = [list(range(0, 8)), list(range(8, 16)), list(range(16, 22))]

_cache = {}


def _ensure_ntff_hook():
    import sys
    import types
    try:
        from antenv.axon_hooks import get_axon_ntff_profile_hook  # noqa: F401
        return
    except ImportError:
        pass
    try:
        mod = types.ModuleType("antenv.axon_hooks")
        mod._hook = None

        def set_axon_ntff_profile_hook(h):
            mod._hook = h

        def get_axon_ntff_profile_hook():
            return mod._hook

        mod.set_axon_ntff_profile_hook = set_axon_ntff_profile_hook
        mod.get_axon_ntff_profile_hook = get_axon_ntff_profile_hook
        sys.modules["antenv.axon_hooks"] = mod
        import antenv
        antenv.axon_hooks = mod
        from trn_agent_boot.trn_boot import _ntff_profile_via_ctypes
        hook = _ntff_profile_via_ctypes("/opt/axon/libaxon_pjrt.so")
        if hook is not None:
            mod.set_axon_ntff_profile_hook(hook)
    except Exception as e:  # profiling is best-effort
        print(f"ntff hook injection failed: {e}")


def _build(a0, a1, qa):
    nc = bacc.Bacc("TRN2", target_bir_lowering=False, debug=False, num_devices=8)

    h0_d = nc.dram_tensor("h0", (C, HW), F16, kind="ExternalInput")
    psrcb_d = nc.dram_tensor("psrcb", (16, 128, 4, 64), F16, kind="ExternalInput")
    gw0_d = nc.dram_tensor("gw0T", (C, C), F16, kind="ExternalInput")
    gw1_d = nc.dram_tensor("gw1T", (C, C), F16, kind="ExternalInput")
    qw1_d = nc.dram_tensor("qw1T", (C, C), F16, kind="ExternalInput")
    qw2_d = nc.dram_tensor("qw2T", (C, C), F16, kind="ExternalInput")
    cw_d = nc.dram_tensor("convwT", (C, 18, C), F16, kind="ExternalInput")
    bias_d = nc.dram_tensor("biases", (C, 4), F32, kind="ExternalInput")
    b1row_d = nc.dram_tensor("b1row", (2, C), F16, kind="ExternalInput")
    ident_d = nc.dram_tensor("ident", (C, C), F16, kind="ExternalInput")
    uvc_d = nc.dram_tensor("uvc", (2, 8, 2816), F16, kind="ExternalInput")
    out_d = nc.dram_tensor("out", (C, HW), F32, kind="ExternalOutput")

    with tile.TileContext(nc) as tc:
        with tc.tile_pool(name="sb", bufs=1) as sb, \
             tc.tile_pool(name="work", bufs=2) as work, \
             tc.tile_pool(name="ps", bufs=1, space="PSUM") as ps, \
             tc.tile_pool(name="dram", bufs=1, space="DRAM") as dram:

            projn_d = dram.tile([8192], F16, tag="projn_d")

            # median block DMAs first so the DVE phase starts immediately
            blks = []
            for g in range(16):
                blk = work.tile([128, 4, 64], F16, tag="blk", bufs=8,
                                name=f"blk_{g}")
                nc.sync.dma_start(blk[:], psrcb_d[g])
                blks.append(blk)

            # ---------------- persistent SBUF ----------------
            h0 = sb.tile([C, 2720], F16, tag="h0")
            nc.sync.dma_start(h0[:, 0:HW], h0_d[:])
            gw0 = sb.tile([C, C], F16, tag="gw0")
            nc.sync.dma_start(gw0[:], gw0_d[:])
            gw1 = sb.tile([C, C], F16, tag="gw1")
            nc.sync.dma_start(gw1[:], gw1_d[:])
            qw1 = sb.tile([C, C], F16, tag="qw1")
            nc.sync.dma_start(qw1[:], qw1_d[:])
            qw2 = sb.tile([C, C], F16, tag="qw2")
            nc.sync.dma_start(qw2[:], qw2_d[:])
            cw = sb.tile([C, 18, C], F16, tag="cw")
            nc.sync.dma_start(cw[:], cw_d[:])
            bia = sb.tile([C, 4], F32, tag="bias")
            nc.sync.dma_start(bia[:], bias_d[:])
            b1row = sb.tile([2, C], F16, tag="b1row")
            nc.sync.dma_start(b1row[:], b1row_d[:])
            ones1 = sb.tile([1, C], F16, tag="ones1")
            ident = sb.tile([C, C], F16, tag="ident")
            nc.sync.dma_start(ident[:], ident_d[:])

            U = sb.tile([8, 2816], F16, tag="U")       # [q; 1]
            nc.sync.dma_start(U[:], uvc_d[0])
            V = sb.tile([8, 2816], F16, tag="V")       # [q; -|p|^2/2]
            nc.sync.dma_start(V[:], uvc_d[1])
            Sbig = sb.tile([C, 22, 2720], F8, tag="Sbig")
            ghrm8 = sb.tile([C, 22, 128], F8, tag="ghrm8")
            M8 = sb.tile([C, 64, 8], F16, tag="M8")
            Mt = sb.tile([64, C], F16, tag="Mt")
            pad0 = sb.tile([C, H + 2, W + 2], F16, tag="pad0")
            pad1 = sb.tile([C, H + 2, W + 2], F16, tag="pad1")
            convacc = sb.tile([C, 2720], F32, tag="convacc")
            ones3 = sb.tile([3, 1], F16, tag="ones3")

            # memsets on gpsimd (DVE stays on the median path)
            nc.gpsimd.memset(Sbig[:, :, HW:HW + 1], 1.0)   # ones-cols for G
            nc.gpsimd.memset(pad0[:], 0.0)
            nc.gpsimd.memset(pad1[:], 0.0)
            nc.gpsimd.memset(ones3[:], 1.0)
            nc.gpsimd.memset(ones1[:], 1.0)

            # ---------------- median pooling + split flatten -----------------
            def median_range(glo, ghi):
                for g in range(glo, ghi):
                    blk = blks[g]
                    for s in range(4):
                        mm8 = work.tile([128, 8], F16, tag="mm8", bufs=8)
                        for rnd in range(3):
                            nc.vector.max(mm8[:], blk[:, s, :])
                            nc.vector.match_replace(blk[:, s, :], mm8[:],
                                                    blk[:, s, :], NEG_F16)
                        nc.vector.max(M8[:, g * 4 + s, :], blk[:, s, :])

            def flatten_half(half):
                lo, nc_ = (0, 32) if half == 0 else (32, 32)
                mtp = ps.tile([C, 1024], F16, tag="sm", bufs=2, name=f"mtp{half}")
                Mcols = M8[:, lo:lo + 32, 7:8].rearrange("p a b -> p (a b)")
                nc.tensor.transpose(mtp[0:32, 0:128], Mcols, ident[:])
                nc.scalar.activation(Mt[lo:lo + 32, :], mtp[0:32, 0:128], AF.Copy)
                projn_r = projn_d.rearrange("(a b) -> a b", b=128)
                nc.sync.dma_start(projn_r[lo:lo + 32, :], Mt[lo:lo + 32, :])

            median_range(0, 8)
            median_range(8, 16)

            # ---------------- iter-1 g-MLP + conv h0-half (under median) -----
            def mlp_layer(w, h_in, out, it, lab, bias, alpha):
                for half, o0, on in ((0, 0, 1536), (1, 1536, HW - 1536)):
                    gp = ps.tile([C, 1536], F32, tag="big3", bufs=2,
                                 name=f"{lab}_{it}_{half}")
                    for c0, ncn in (CHUNKS6[:3] if half == 0 else CHUNKS6[3:]):
                        nc.tensor.matmul(gp[:, c0 - o0:c0 - o0 + ncn], w[:],
                                         h_in[:, c0:c0 + ncn], start=True, stop=True)
                    nc.scalar.activation(out[:, o0:o0 + on], gp[:, 0:on], AF.Prelu,
                                         bias=bias, alpha=alpha)

            def gmlp_t(h_in, it):
                """g-MLP: layer 1 straight, layer 2 in transposed orientation
                writing gh2^T tiles directly into ghrm8 (fp8). The b1 bias is
                added via a rank-1 matmul (per-feature = free dim here)."""
                gh1 = work.tile([C, 2720], F16, tag="gh", bufs=1, name=f"gh1_{it}")
                mlp_layer(gw0, h_in, gh1, it, "g1", bia[:, 0:1], a0)
                for jt, (j0, nj) in enumerate(PTILES):
                    lp32 = ps.tile([C, 512], F32, tag="sm", bufs=2,
                                   name=f"l2t_{it}_{jt}")
                    nc.tensor.matmul(lp32[0:nj, 0:128], gh1[:, j0:j0 + nj],
                                     gw1[:], start=True, stop=False)
                    nc.tensor.matmul(lp32[0:nj, 0:128], ones1[0:1, 0:nj],
                                     b1row[0:1, :], start=False, stop=True)
                    nc.scalar.activation(ghrm8[0:nj, jt, :], lp32[0:nj, 0:128],
                                         AF.Prelu, alpha=a1)

            gmlp_t(h0, 0)

            nc.scalar.activation(pad0[:, 1:H + 1, 1:W + 1],
                                 h0[:, 0:HW].rearrange("p (h w) -> p h w", h=H), AF.Copy)
            taps = [(a, b) for a in range(3) for b in range(3)]
            for ri, (r0, nr) in enumerate(RCHUNKS):
                cpe = ps.tile([C, 512], F32, tag="sm", bufs=2, name=f"cpe_{ri}")
                for ti, (dy, dx) in enumerate(taps):
                    idx = (dy * 3 + dx) * 2
                    nc.tensor.matmul(cpe[:, 0:nr * W], cw[:, idx, :],
                                     pad0[:, r0 + dy:r0 + dy + nr, dx:dx + W],
                                     start=(ti == 0), stop=(ti == 8))
                nc.scalar.activation(convacc[:, r0 * W:(r0 + nr) * W],
                                     cpe[:, 0:nr * W], AF.Identity, bias=bia[:, 3:4])

            # ---------------- proj flatten + U/V staging ---------------------
            flatten_half(0)
            # x channel DMA overlaps the second median half
            nc.sync.dma_start(V[0:1, 0:HW], projn_d[0:HW])
            sq3 = work.tile([3, 2720], F16, tag="sq3", bufs=1, name="sq3")

            nc.sync.dma_start(U[0:1, 0:HW], projn_d[0:HW])
            flatten_half(1)
            for ch in (1, 2):
                nc.sync.dma_start(V[ch:ch + 1, 0:HW], projn_d[ch * HW:(ch + 1) * HW])
                nc.sync.dma_start(U[ch:ch + 1, 0:HW], projn_d[ch * HW:(ch + 1) * HW])
            # e'' = q.p - |p_j|^2/2 (same order as e' = 2q.p - |p_j|^2)
            nc.vector.tensor_tensor(sq3[0:3, 0:HW], V[0:3, 0:HW], V[0:3, 0:HW],
                                    ALU.mult)
            sqp = ps.tile([C, 1536], F32, tag="big3", bufs=2, name="sqp")
            for c0, ncn in CHUNKS6[:3]:
                nc.tensor.matmul(sqp[0:1, c0:c0 + ncn], ones3[:],
                                 sq3[:, c0:c0 + ncn], start=True, stop=True)
            hirow = work.tile([1, 2816], F16, tag="row", bufs=1, name="hirow")
            nc.scalar.activation(hirow[0:1, 0:1536], sqp[0:1, 0:1536],
                                 AF.Copy, scale=-0.5)
            nc.sync.dma_start(V[3:4, 0:1536], hirow[0:1, 0:1536])
            sqp2 = ps.tile([C, 1536], F32, tag="big3", bufs=2, name="sqp2")
            for c0, ncn in CHUNKS6[3:]:
                nc.tensor.matmul(sqp2[0:1, c0 - 1536:c0 - 1536 + ncn],
                                 ones3[:], sq3[:, c0:c0 + ncn], start=True, stop=True)
            nc.scalar.activation(hirow[0:1, 1536:HW], sqp2[0:1, 0:HW - 1536],
                                 AF.Copy, scale=-0.5)
            nc.sync.dma_start(V[3:4, 1536:HW], hirow[0:1, 1536:HW])

            # ---------------- p1: per-row te + sign, software-pipelined ------
            efs = {}

            def stage_ef(jt):
                i0, ni = PTILES[jt]
                ef = work.tile([C, 2720], F16, tag="ef", bufs=4, name=f"ef_{jt}")
                efs[jt] = ef
                for half, o0, on in ((0, 0, 1536), (1, 1536, HW - 1536)):
                    pp = ps.tile([C, 1536], F32, tag="big3", bufs=2,
                                 name=f"pp_{jt}_{half}")
                    for c0, ncn in (CHUNKS6[:3] if half == 0 else CHUNKS6[3:]):
                        nc.tensor.matmul(pp[0:ni, c0 - o0:c0 - o0 + ncn],
                                         U[0:4, i0:i0 + ni], V[0:4, c0:c0 + ncn],
                                         start=True, stop=True)
                    nc.scalar.activation(ef[0:ni, o0:o0 + on], pp[0:ni, 0:on],
                                         AF.Copy)

            def p1_scan(jt):
                i0, ni = PTILES[jt]
                ef = efs[jt]
                t8a = work.tile([C, 8], F16, tag="t8", bufs=6, name=f"t8a_{jt}")
                nc.vector.max(t8a[0:ni], ef[0:ni, 0:HW])
                v8f = work.tile([C, 1], F32, tag="v8f", bufs=12, name=f"v8f_{jt}")
                nc.vector.tensor_copy(v8f[0:ni], t8a[0:ni, 7:8])
                msk = work.tile([C, 2720], F16, tag="msk", bufs=3, name=f"msk_{jt}")
                nc.vector.tensor_scalar(msk[0:ni, 0:HW], ef[0:ni, 0:HW],
                                        v8f[0:ni], NEG_F16,
                                        op0=ALU.is_ge, op1=ALU.mult)
                nc.vector.tensor_tensor(msk[0:ni, 0:HW], ef[0:ni, 0:HW],
                                        msk[0:ni, 0:HW], ALU.add)
                t8b = work.tile([C, 8], F16, tag="t8", bufs=6, name=f"t8b_{jt}")
                nc.vector.max(t8b[0:ni], msk[0:ni, 0:HW])
                # bias = -te + |te|*2^-11 + 4e-7
                tp1 = work.tile([C, 1], F32, tag="v8f", bufs=12, name=f"tp1_{jt}")
                nc.vector.tensor_scalar(tp1[0:ni], t8b[0:ni, 7:8], 2.0 ** -11, 0.0,
                                        op0=ALU.mult, op1=ALU.add)
                tab = work.tile([C, 1], F32, tag="v8f", bufs=12, name=f"tab_{jt}")
                nc.vector.scalar_tensor_tensor(tab[0:ni], t8b[0:ni, 7:8],
                                               -(2.0 ** -11), tp1[0:ni],
                                               ALU.mult, ALU.max)
                bv = work.tile([C, 1], F32, tag="v8f", bufs=12, name=f"bv_{jt}")
                nc.vector.scalar_tensor_tensor(bv[0:ni], tab[0:ni], 4.0e-7,
                                               t8b[0:ni, 7:8], ALU.add, ALU.subtract)
                return bv

            def p1_sign(jt, bv):
                i0, ni = PTILES[jt]
                nc.scalar.activation(Sbig[0:ni, jt, 0:HW], efs[jt][0:ni, 0:HW],
                                     AF.Sign, bias=bv[0:ni])

            stage_ef(0)
            stage_ef(1)
            for jt in range(22):
                bv = p1_scan(jt)
                if jt + 2 < 22:
                    stage_ef(jt + 2)
                p1_sign(jt, bv)

            # ---------------- dense tail: agg1+q1, gmlp2, agg2+q2, conv ------
            DR = mybir.MatmulPerfMode.DoubleRow

            def agg_chunk(tgt, c0, ncn, first):
                # pairs of full 128-row tiles via fp8 DoubleRow (2 k-tiles per
                # matmul), then tiles 20 (128 rows) and 21 (12 rows) normally
                for pr in range(10):
                    nc.tensor.matmul(tgt,
                                     ghrm8[:, 2 * pr:2 * pr + 2, :],
                                     Sbig[:, 2 * pr:2 * pr + 2, c0:c0 + ncn],
                                     start=(pr == 0), stop=False,
                                     perf_mode=DR)
                for jt in (20, 21):
                    j0, nj = PTILES[jt]
                    nc.tensor.matmul(tgt,
                                     ghrm8[0:nj, jt, :],
                                     Sbig[0:nj, jt, c0:c0 + ncn],
                                     start=False, stop=(jt == 21))

            def q_half(qp, h_in, mts, half, o0):
                for c0, ncn in (CHUNKS6[:3] if half == 0 else CHUNKS6[3:]):
                    nc.tensor.matmul(qp[:, c0 - o0:c0 - o0 + ncn], qw1[:],
                                     h_in[:, c0:c0 + ncn], start=True, stop=False)
                    nc.tensor.matmul(qp[:, c0 - o0:c0 - o0 + ncn], qw2[:],
                                     mts[:, c0 - o0:c0 - o0 + ncn],
                                     start=False, stop=True)

            def agg_q(it, h_in, pad=None):
                """Aggregation (G-chunk first) fused with the q update so the
                q matmuls overlap the remaining aggregation chunks."""
                A = ps.tile([C, 1536], F32, tag="big3", bufs=2, name=f"agg{it}A")
                B = ps.tile([C, 1536], F32, tag="big3", bufs=2, name=f"agg{it}B")
                agg_chunk(B[:, 1024:1165], 2560, 141, True)
                gcol = sb.tile([C, 1], F32, tag=f"gcol_{it}")
                nc.scalar.activation(gcol[:], B[:, 1164:1165], AF.Copy)
                for c0 in (0, 512, 1024):
                    agg_chunk(A[:, c0:c0 + 512], c0, 512, False)
                mtsA = work.tile([C, 1536], F16, tag="mtsA", bufs=1,
                                 name=f"mtsA_{it}")
                nc.scalar.activation(mtsA[:], A[:, 0:1536], AF.Identity,
                                     bias=gcol[:])
                agg_chunk(B[:, 0:512], 1536, 512, False)
                h_out = work.tile([C, 2720], F16, tag="h", bufs=2, name=f"h_{it}")
                qpA = ps.tile([C, 1536], F32, tag="big3", bufs=2,
                              name=f"qp_{it}_0")
                q_half(qpA, h_in, mtsA, 0, 0)
                nc.scalar.activation(h_out[:, 0:1536], qpA[:, 0:1536], AF.Prelu,
                                     bias=bia[:, 2:3], alpha=qa)
                if pad is not None:
                    nc.scalar.activation(
                        pad[:, 1:26, 1:W + 1],
                        h_out[:, 0:1500].rearrange("p (h w) -> p h w", w=W),
                        AF.Copy)
                agg_chunk(B[:, 512:1024], 2048, 512, False)
                mtsB = work.tile([C, 1536], F16, tag="mtsB", bufs=1,
                                 name=f"mtsB_{it}")
                nc.scalar.activation(mtsB[:, 0:1164], B[:, 0:1164], AF.Identity,
                                     bias=gcol[:])
                qpB = ps.tile([C, 1536], F32, tag="big3", bufs=2,
                              name=f"qp_{it}_1")
                q_half(qpB, h_in, mtsB, 1, 1536)
                nc.scalar.activation(h_out[:, 1536:HW], qpB[:, 0:HW - 1536],
                                     AF.Prelu, bias=bia[:, 2:3], alpha=qa)
                if pad is not None:
                    nc.scalar.activation(
                        pad[:, 26:H + 1, 1:W + 1],
                        h_out[:, 1500:HW].rearrange("p (h w) -> p h w", w=W),
                        AF.Copy)
                return h_out

            h1 = agg_q(0, h0)
            gmlp_t(h1, 1)
            h2 = agg_q(1, h1, pad=pad1)

            oc = work.tile([C, 2720], F32, tag="bigf32", bufs=1, name="oc")
            for ri, (r0, nr) in enumerate(RCHUNKS):
                cpe = ps.tile([C, 512], F32, tag="sm", bufs=2, name=f"cp2_{ri}")
                for ti, (dy, dx) in enumerate(taps):
                    idx = (dy * 3 + dx) * 2 + 1
                    nc.tensor.matmul(cpe[:, 0:nr * W], cw[:, idx, :],
                                     pad1[:, r0 + dy:r0 + dy + nr, dx:dx + W],
                                     start=(ti == 0), stop=(ti == 8))
                nc.vector.tensor_tensor(oc[:, r0 * W:(r0 + nr) * W],
                                        cpe[:, 0:nr * W],
                                        convacc[:, r0 * W:(r0 + nr) * W], ALU.add)
                if ri == 2:
                    nc.sync.dma_start(out_d[:, 0:1440], oc[:, 0:1440])
                elif ri == 5:
                    nc.sync.dma_start(out_d[:, 1440:2700], oc[:, 1440:2700])

    nc.compile()
    return nc


def kernel(cnn_encoder_output, original_input, xy,
           g_w0, g_b0, g_a0, g_w1, g_b1, g_a1,
           q_w, q_b, q_a, conv_w, conv_b,
           gnn_iterations, k, use_half_precision, _trace=False):
    assert int(gnn_iterations) == 2 and int(k) == 16 and int(use_half_precision) == 0

    cnn = np.asarray(cnn_encoder_output, dtype=np.float32)
    orig = np.asarray(original_input, dtype=np.float32)
    xy = np.asarray(xy, dtype=np.float32)
    a0, a1, qa = float(np.ravel(g_a0)[0]), float(np.ravel(g_a1)[0]), float(np.ravel(q_a)[0])

    key = (a0, a1, qa)
    if key not in _cache:
        _cache[key] = _build(a0, a1, qa)
    nc = _cache[key]

    g_w0 = np.asarray(g_w0, np.float32)
    g_w1 = np.asarray(g_w1, np.float32)
    q_w = np.asarray(q_w, np.float32)
    conv_w = np.asarray(conv_w, np.float32)

    gw0T = np.ascontiguousarray(g_w0.T).astype(np.float16)
    gw1T = np.ascontiguousarray(g_w1.T).astype(np.float16)
    qw1T = np.ascontiguousarray(q_w[:, :C].T).astype(np.float16)
    qw2T = np.ascontiguousarray(q_w[:, C:].T / float(2 * K)).astype(np.float16)
    cwT = np.empty((C, 18, C), np.float16)
    for dy in range(3):
        for dx in range(3):
            for kh in range(2):
                idx = (dy * 3 + dx) * 2 + kh
                cwT[:, idx, :] = conv_w[:, kh * C:(kh + 1) * C, dy, dx].T.astype(np.float16)
    biases = np.stack([np.asarray(g_b0, np.float32), np.asarray(g_b1, np.float32),
                       np.asarray(q_b, np.float32), np.asarray(conv_b, np.float32)],
                      axis=1)
    b1row = np.stack([np.asarray(g_b1, np.float16),
                      np.asarray(g_b0, np.float16)], axis=0)
    ident = np.eye(C, dtype=np.float16)
    uvc = np.zeros((2, 8, 2816), np.float16)
    uvc[0, 3] = 1.0

    shared = dict(gw0T=gw0T, gw1T=gw1T, qw1T=qw1T, qw2T=qw2T, convwT=cwT,
                  biases=np.ascontiguousarray(biases),
                  b1row=np.ascontiguousarray(b1row), ident=ident, uvc=uvc)
    in_maps = []
    for n in range(N):
        chans = np.stack([xy[n, 0], xy[n, 1], orig[n, 3]], axis=0)      # [3, 360, 480]
        blocks = chans.reshape(3, H, 8, W, 8).transpose(0, 1, 3, 2, 4).reshape(3 * HW, 64)
        blocks = (-blocks).astype(np.float16)
        pad = np.zeros((8192, 64), np.float16)
        pad[:3 * HW] = blocks
        psrcb = pad.reshape(16, 4, 128, 64).transpose(0, 2, 1, 3)
        in_maps.append(dict(h0=np.ascontiguousarray(
                                cnn[n].reshape(C, HW).astype(np.float16)),
                            psrcb=np.ascontiguousarray(psrcb), **shared))

    if _trace:
        _ensure_ntff_hook()
    res = run_bass_kernel_spmd(nc, in_maps, core_ids=list(range(N)), trace=_trace,
                               trace_cores=list(range(N)) if _trace else None)
    out = np.stack([res.results[n]["out"].reshape(C, H, W).astype(np.float32)
                    for n in range(N)])
    if _trace:
        kernel._last_results = res
    return out


# revision 38
# speedup vs baseline: 1.1272x; 1.0802x over previous
"""Trainium2 Bass kernel for EnetGnn (gnn_message_passing).

Data-parallel over batch N=8, one sample per NeuronCore. Per-core design:

1. Median pool: host stages negated fp16 blocks in [16, 128, 4, 64] tiles so
   each load is one contiguous 64KB DMA. DVE max8/match_replace rank-32
   rounds; medians flattened via two half PE transposes + DMA so the x
   channel stages while the second half of the median still runs.
2. KNN threshold: e''[i,j] = p_i.p_j - |p_j|^2/2 (same order as the full
   e') via K=4 fp16 matmuls into double-buffered 3-bank psum halves,
   ACT-evacuated to fp16 ef two tiles ahead of the scans. Per-row
   16th-largest te via max8 + is_ge mask removal + max8 (all DVE, ~8.7us
   per 128-row tile; DVE is the saturated critical engine here).
3. S = Sign(ef - te + eps) on the ACT engine with per-row bias, emitted one
   tile behind the DVE scan so the ACT FIFO never blocks the next tile's
   psum evacuation. S tiles land in one [128, 22, 2720] fp8 SBUF tensor
   with a ones-column that accumulates G = sum_j gh_j.
4. All heavy matmul work runs as one dense tail stream to keep the PE at
   its hot clock: aggregation in fp8 DoubleRow pairs (2 k-tiles per
   instruction) with the G column folded into the mts evacuation as a
   per-partition ACT bias; mts in fp16 keeps the q update all-fp16 (fp32
   moving operands stream 4x slower). The g-MLP's second layer is computed
   in transposed orientation straight into ghrm8 (bias via rank-1 matmul),
   so no PE transposes sit between the MLP and the aggregation.
"""
import numpy as np
import concourse.bass as bass
import concourse.bacc as bacc
import concourse.mybir as mybir
import concourse.tile as tile
from concourse.bass_utils import run_bass_kernel_spmd

F32 = mybir.dt.float32
F16 = mybir.dt.float16
F8 = mybir.dt.float8e4
AF = mybir.ActivationFunctionType
ALU = mybir.AluOpType

N, C, H, W = 8, 128, 45, 60
HW = H * W                      # 2700
K = 16
NEG_F16 = -60000.0

CHUNKS6 = [(0, 512), (512, 512), (1024, 512), (1536, 512), (2048, 512), (2560, 140)]
AGG_CHUNKS = [(0, 512), (512, 512), (1024, 512), (1536, 512), (2048, 512), (2560, 141)]
PTILES = [(t * 128, 128) for t in range(21)] + [(2688, 12)]
RCHUNKS = [(0, 8), (8, 8), (16, 8), (24, 8), (32, 8), (40, 5)]

_cache = {}


def _ensure_ntff_hook():
    import sys
    import types
    try:
        from antenv.axon_hooks import get_axon_ntff_profile_hook  # noqa: F401
        return
    except ImportError:
        pass
    try:
        mod = types.ModuleType("antenv.axon_hooks")
        mod._hook = None

        def set_axon_ntff_profile_hook(h):
            mod._hook = h

        def get_axon_ntff_profile_hook():
            return mod._hook

        mod.set_axon_ntff_profile_hook = set_axon_ntff_profile_hook
        mod.get_axon_ntff_profile_hook = get_axon_ntff_profile_hook
        sys.modules["antenv.axon_hooks"] = mod
        import antenv
        antenv.axon_hooks = mod
        from trn_agent_boot.trn_boot import _ntff_profile_via_ctypes
        hook = _ntff_profile_via_ctypes("/opt/axon/libaxon_pjrt.so")
        if hook is not None:
            mod.set_axon_ntff_profile_hook(hook)
    except Exception as e:  # profiling is best-effort
        print(f"ntff hook injection failed: {e}")


def _build(a0, a1, qa):
    nc = bacc.Bacc("TRN2", target_bir_lowering=False, debug=False, num_devices=8)

    h0_d = nc.dram_tensor("h0", (C, HW), F16, kind="ExternalInput")
    psrcb_d = nc.dram_tensor("psrcb", (16, 128, 4, 64), F16, kind="ExternalInput")
    gw0_d = nc.dram_tensor("gw0T", (C, C), F16, kind="ExternalInput")
    gw1_d = nc.dram_tensor("gw1T", (C, C), F16, kind="ExternalInput")
    qw1_d = nc.dram_tensor("qw1T", (C, C), F16, kind="ExternalInput")
    qw2_d = nc.dram_tensor("qw2T", (C, C), F16, kind="ExternalInput")
    cw_d = nc.dram_tensor("convwT", (C, 18, C), F16, kind="ExternalInput")
    bias_d = nc.dram_tensor("biases", (C, 4), F32, kind="ExternalInput")
    b1row_d = nc.dram_tensor("b1row", (2, C), F16, kind="ExternalInput")
    ident_d = nc.dram_tensor("ident", (C, C), F16, kind="ExternalInput")
    uvc_d = nc.dram_tensor("uvc", (2, 8, 2816), F16, kind="ExternalInput")
    out_d = nc.dram_tensor("out", (C, HW), F32, kind="ExternalOutput")

    with tile.TileContext(nc) as tc:
        with tc.tile_pool(name="sb", bufs=1) as sb, \
             tc.tile_pool(name="work", bufs=2) as work, \
             tc.tile_pool(name="ps", bufs=1, space="PSUM") as ps, \
             tc.tile_pool(name="dram", bufs=1, space="DRAM") as dram:

            projn_d = dram.tile([8192], F16, tag="projn_d")

            # median block DMAs first so the DVE phase starts immediately
            blks = []
            for g in range(16):
                blk = work.tile([128, 4, 64], F16, tag="blk", bufs=8,
                                name=f"blk_{g}")
                nc.sync.dma_start(blk[:], psrcb_d[g])
                blks.append(blk)

            # ---------------- persistent SBUF ----------------
            h0 = sb.tile([C, 2720], F16, tag="h0")
            nc.sync.dma_start(h0[:, 0:HW], h0_d[:])
            gw0 = sb.tile([C, C], F16, tag="gw0")
            nc.sync.dma_start(gw0[:], gw0_d[:])
            gw1 = sb.tile([C, C], F16, tag="gw1")
            nc.sync.dma_start(gw1[:], gw1_d[:])
            qw1 = sb.tile([C, C], F16, tag="qw1")
            nc.sync.dma_start(qw1[:], qw1_d[:])
            qw2 = sb.tile([C, C], F16, tag="qw2")
            nc.sync.dma_start(qw2[:], qw2_d[:])
            cw = sb.tile([C, 18, C], F16, tag="cw")
            nc.sync.dma_start(cw[:], cw_d[:])
            bia = sb.tile([C, 4], F32, tag="bias")
            nc.sync.dma_start(bia[:], bias_d[:])
            b1row = sb.tile([2, C], F16, tag="b1row")
            nc.sync.dma_start(b1row[:], b1row_d[:])
            ones1 = sb.tile([1, C], F16, tag="ones1")
            ident = sb.tile([C, C], F16, tag="ident")
            nc.sync.dma_start(ident[:], ident_d[:])

            U = sb.tile([8, 2816], F16, tag="U")       # [q; 1]
            nc.sync.dma_start(U[:], uvc_d[0])
            V = sb.tile([8, 2816], F16, tag="V")       # [q; -|p|^2/2]
            nc.sync.dma_start(V[:], uvc_d[1])
            Sbig = sb.tile([C, 22, 2720], F8, tag="Sbig")
            ghrm8 = sb.tile([C, 22, 128], F8, tag="ghrm8")
            M8 = sb.tile([C, 64, 8], F16, tag="M8")
            Mt = sb.tile([64, C], F16, tag="Mt")
            pad0 = sb.tile([C, H + 2, W + 2], F16, tag="pad0")
            pad1 = sb.tile([C, H + 2, W + 2], F16, tag="pad1")
            convacc = sb.tile([C, 2720], F32, tag="convacc")
            ones3 = sb.tile([3, 1], F16, tag="ones3")

            # memsets on gpsimd (DVE stays on the median path)
            nc.gpsimd.memset(Sbig[:, :, HW:HW + 1], 1.0)   # ones-cols for G
            nc.gpsimd.memset(pad0[:], 0.0)
            nc.gpsimd.memset(pad1[:], 0.0)
            nc.gpsimd.memset(ones3[:], 1.0)
            nc.gpsimd.memset(ones1[:], 1.0)

            # ---------------- median pooling + split flatten -----------------
            def median_range(glo, ghi):
                for g in range(glo, ghi):
                    blk = blks[g]
                    for s in range(4):
                        mm8 = work.tile([128, 8], F16, tag="mm8", bufs=8)
                        for rnd in range(3):
                            nc.vector.max(mm8[:], blk[:, s, :])
                            nc.vector.match_replace(blk[:, s, :], mm8[:],
                                                    blk[:, s, :], NEG_F16)
                        nc.vector.max(M8[:, g * 4 + s, :], blk[:, s, :])

            def flatten_half(half):
                lo, nc_ = (0, 32) if half == 0 else (32, 32)
                mtp = ps.tile([C, 1024], F16, tag="sm", bufs=2, name=f"mtp{half}")
                Mcols = M8[:, lo:lo + 32, 7:8].rearrange("p a b -> p (a b)")
                nc.tensor.transpose(mtp[0:32, 0:128], Mcols, ident[:])
                nc.scalar.activation(Mt[lo:lo + 32, :], mtp[0:32, 0:128], AF.Copy)
                projn_r = projn_d.rearrange("(a b) -> a b", b=128)
                nc.sync.dma_start(projn_r[lo:lo + 32, :], Mt[lo:lo + 32, :])

            median_range(0, 8)
            median_range(8, 16)

            # ---------------- iter-1 g-MLP + conv h0-half (under median) -----
            def mlp_layer(w, h_in, out, it, lab, bias, alpha):
                for half, o0, on in ((0, 0, 1536), (1, 1536, HW - 1536)):
                    gp = ps.tile([C, 1536], F32, tag="big3", bufs=2,
                                 name=f"{lab}_{it}_{half}")
                    for c0, ncn in (CHUNKS6[:3] if half == 0 else CHUNKS6[3:]):
                        nc.tensor.matmul(gp[:, c0 - o0:c0 - o0 + ncn], w[:],
                                         h_in[:, c0:c0 + ncn], start=True, stop=True)
                    nc.scalar.activation(out[:, o0:o0 + on], gp[:, 0:on], AF.Prelu,
                                         bias=bias, alpha=alpha)

            def gmlp_t(h_in, it):
                """g-MLP: layer 1 straight, layer 2 in transposed orientation
                writing gh2^T tiles directly into ghrm8 (fp8). The b1 bias is
                added via a rank-1 matmul (per-feature = free dim here)."""
                gh1 = work.tile([C, 2720], F16, tag="gh", bufs=1, name=f"gh1_{it}")
                mlp_layer(gw0, h_in, gh1, it, "g1", bia[:, 0:1], a0)
                for jt, (j0, nj) in enumerate(PTILES):
                    lp32 = ps.tile([C, 512], F32, tag="sm", bufs=2,
                                   name=f"l2t_{it}_{jt}")
                    nc.tensor.matmul(lp32[0:nj, 0:128], gh1[:, j0:j0 + nj],
                                     gw1[:], start=True, stop=False)
                    nc.tensor.matmul(lp32[0:nj, 0:128], ones1[0:1, 0:nj],
                                     b1row[0:1, :], start=False, stop=True)
                    nc.scalar.activation(ghrm8[0:nj, jt, :], lp32[0:nj, 0:128],
                                         AF.Prelu, alpha=a1)

            gmlp_t(h0, 0)

            nc.scalar.activation(pad0[:, 1:H + 1, 1:W + 1],
                                 h0[:, 0:HW].rearrange("p (h w) -> p h w", h=H), AF.Copy)
            taps = [(a, b) for a in range(3) for b in range(3)]
            for ri, (r0, nr) in enumerate(RCHUNKS):
                cpe = ps.tile([C, 512], F32, tag="sm", bufs=2, name=f"cpe_{ri}")
                for ti, (dy, dx) in enumerate(taps):
                    idx = (dy * 3 + dx) * 2
                    nc.tensor.matmul(cpe[:, 0:nr * W], cw[:, idx, :],
                                     pad0[:, r0 + dy:r0 + dy + nr, dx:dx + W],
                                     start=(ti == 0), stop=(ti == 8))
                nc.scalar.activation(convacc[:, r0 * W:(r0 + nr) * W],
                                     cpe[:, 0:nr * W], AF.Identity, bias=bia[:, 3:4])

            # ---------------- proj flatten + U/V staging ---------------------
            flatten_half(0)
            # x channel DMA overlaps the second median half
            nc.sync.dma_start(V[0:1, 0:HW], projn_d[0:HW])
            sq3 = work.tile([3, 2720], F16, tag="sq3", bufs=1, name="sq3")

            nc.sync.dma_start(U[0:1, 0:HW], projn_d[0:HW])
            flatten_half(1)
            for ch in (1, 2):
                nc.sync.dma_start(V[ch:ch + 1, 0:HW], projn_d[ch * HW:(ch + 1) * HW])
                nc.sync.dma_start(U[ch:ch + 1, 0:HW], projn_d[ch * HW:(ch + 1) * HW])
            # e'' = q.p - |p_j|^2/2 (same order as e' = 2q.p - |p_j|^2)
            nc.vector.tensor_tensor(sq3[0:3, 0:HW], V[0:3, 0:HW], V[0:3, 0:HW],
                                    ALU.mult)
            sqp = ps.tile([C, 1536], F32, tag="big3", bufs=2, name="sqp")
            for c0, ncn in CHUNKS6[:3]:
                nc.tensor.matmul(sqp[0:1, c0:c0 + ncn], ones3[:],
                                 sq3[:, c0:c0 + ncn], start=True, stop=True)
            hirow = work.tile([1, 2816], F16, tag="row", bufs=1, name="hirow")
            nc.scalar.activation(hirow[0:1, 0:1536], sqp[0:1, 0:1536],
                                 AF.Copy, scale=-0.5)
            nc.sync.dma_start(V[3:4, 0:1536], hirow[0:1, 0:1536])
            sqp2 = ps.tile([C, 1536], F32, tag="big3", bufs=2, name="sqp2")
            for c0, ncn in CHUNKS6[3:]:
                nc.tensor.matmul(sqp2[0:1, c0 - 1536:c0 - 1536 + ncn],
                                 ones3[:], sq3[:, c0:c0 + ncn], start=True, stop=True)
            nc.scalar.activation(hirow[0:1, 1536:HW], sqp2[0:1, 0:HW - 1536],
                                 AF.Copy, scale=-0.5)
            nc.sync.dma_start(V[3:4, 1536:HW], hirow[0:1, 1536:HW])

            # ---------------- p1: per-row te + sign, software-pipelined ------
            efs = {}

            def stage_ef(jt):
                i0, ni = PTILES[jt]
                ef = work.tile([C, 2720], F16, tag="ef", bufs=4, name=f"ef_{jt}")
                efs[jt] = ef
                for half, o0, on in ((0, 0, 1536), (1, 1536, HW - 1536)):
                    pp = ps.tile([C, 1536], F32, tag="big3", bufs=2,
                                 name=f"pp_{jt}_{half}")
                    for c0, ncn in (CHUNKS6[:3] if half == 0 else CHUNKS6[3:]):
                        nc.tensor.matmul(pp[0:ni, c0 - o0:c0 - o0 + ncn],
                                         U[0:4, i0:i0 + ni], V[0:4, c0:c0 + ncn],
                                         start=True, stop=True)
                    nc.scalar.activation(ef[0:ni, o0:o0 + on], pp[0:ni, 0:on],
                                         AF.Copy)

            def p1_scan(jt):
                i0, ni = PTILES[jt]
                ef = efs[jt]
                t8a = work.tile([C, 8], F16, tag="t8", bufs=6, name=f"t8a_{jt}")
                nc.vector.max(t8a[0:ni], ef[0:ni, 0:HW])
                v8f = work.tile([C, 1], F32, tag="v8f", bufs=12, name=f"v8f_{jt}")
                nc.vector.tensor_copy(v8f[0:ni], t8a[0:ni, 7:8])
                msk = work.tile([C, 2720], F16, tag="msk", bufs=3, name=f"msk_{jt}")
                nc.vector.tensor_scalar(msk[0:ni, 0:HW], ef[0:ni, 0:HW],
                                        v8f[0:ni], NEG_F16,
                                        op0=ALU.is_ge, op1=ALU.mult)
                nc.vector.tensor_tensor(msk[0:ni, 0:HW], ef[0:ni, 0:HW],
                                        msk[0:ni, 0:HW], ALU.add)
                t8b = work.tile([C, 8], F16, tag="t8", bufs=6, name=f"t8b_{jt}")
                nc.vector.max(t8b[0:ni], msk[0:ni, 0:HW])
                # bias = -te + |te|*2^-11 + 4e-7
                tp1 = work.tile([C, 1], F32, tag="v8f", bufs=12, name=f"tp1_{jt}")
                nc.vector.tensor_scalar(tp1[0:ni], t8b[0:ni, 7:8], 2.0 ** -11, 0.0,
                                        op0=ALU.mult, op1=ALU.add)
                tab = work.tile([C, 1], F32, tag="v8f", bufs=12, name=f"tab_{jt}")
                nc.vector.scalar_tensor_tensor(tab[0:ni], t8b[0:ni, 7:8],
                                               -(2.0 ** -11), tp1[0:ni],
                                               ALU.mult, ALU.max)
                bv = work.tile([C, 1], F32, tag="v8f", bufs=12, name=f"bv_{jt}")
                nc.vector.scalar_tensor_tensor(bv[0:ni], tab[0:ni], 4.0e-7,
                                               t8b[0:ni, 7:8], ALU.add, ALU.subtract)
                return bv

            def p1_sign(jt, bv):
                i0, ni = PTILES[jt]
                nc.scalar.activation(Sbig[0:ni, jt, 0:HW], efs[jt][0:ni, 0:HW],
                                     AF.Sign, bias=bv[0:ni])

            stage_ef(0)
            stage_ef(1)
            for jt in range(22):
                bv = p1_scan(jt)
                if jt + 2 < 22:
                    stage_ef(jt + 2)
                p1_sign(jt, bv)

            # ---------------- dense tail: agg1+q1, gmlp2, agg2+q2, conv ------
            DR = mybir.MatmulPerfMode.DoubleRow

            def agg_chunk(tgt, c0, ncn, first):
                # pairs of full 128-row tiles via fp8 DoubleRow (2 k-tiles per
                # matmul), then tiles 20 (128 rows) and 21 (12 rows) normally
                for pr in range(10):
                    nc.tensor.matmul(tgt,
                                     ghrm8[:, 2 * pr:2 * pr + 2, :],
                                     Sbig[:, 2 * pr:2 * pr + 2, c0:c0 + ncn],
                                     start=(pr == 0), stop=False,
                                     perf_mode=DR)
                for jt in (20, 21):
                    j0, nj = PTILES[jt]
                    nc.tensor.matmul(tgt,
                                     ghrm8[0:nj, jt, :],
                                     Sbig[0:nj, jt, c0:c0 + ncn],
                                     start=False, stop=(jt == 21))

            def q_half(qp, h_in, mts, half, o0):
                for c0, ncn in (CHUNKS6[:3] if half == 0 else CHUNKS6[3:]):
                    nc.tensor.matmul(qp[:, c0 - o0:c0 - o0 + ncn], qw1[:],
                                     h_in[:, c0:c0 + ncn], start=True, stop=False)
                    nc.tensor.matmul(qp[:, c0 - o0:c0 - o0 + ncn], qw2[:],
                                     mts[:, c0 - o0:c0 - o0 + ncn],
                                     start=False, stop=True)

            def agg_q(it, h_in, pad=None):
                """Aggregation (G-chunk first) fused with the q update so the
                q matmuls overlap the remaining aggregation chunks."""
                A = ps.tile([C, 1536], F32, tag="big3", bufs=2, name=f"agg{it}A")
                B = ps.tile([C, 1536], F32, tag="big3", bufs=2, name=f"agg{it}B")
                agg_chunk(B[:, 1024:1165], 2560, 141, True)
                gcol = sb.tile([C, 1], F32, tag=f"gcol_{it}")
                nc.scalar.activation(gcol[:], B[:, 1164:1165], AF.Copy)
                for c0 in (0, 512, 1024):
                    agg_chunk(A[:, c0:c0 + 512], c0, 512, False)
                mtsA = work.tile([C, 1536], F16, tag="mtsA", bufs=1,
                                 name=f"mtsA_{it}")
                nc.scalar.activation(mtsA[:], A[:, 0:1536], AF.Identity,
                                     bias=gcol[:])
                agg_chunk(B[:, 0:512], 1536, 512, False)
                h_out = work.tile([C, 2720], F16, tag="h", bufs=2, name=f"h_{it}")
                qpA = ps.tile([C, 1536], F32, tag="big3", bufs=2,
                              name=f"qp_{it}_0")
                q_half(qpA, h_in, mtsA, 0, 0)
                nc.scalar.activation(h_out[:, 0:1536], qpA[:, 0:1536], AF.Prelu,
                                     bias=bia[:, 2:3], alpha=qa)
                if pad is not None:
                    nc.scalar.activation(
                        pad[:, 1:26, 1:W + 1],
                        h_out[:, 0:1500].rearrange("p (h w) -> p h w", w=W),
                        AF.Copy)
                agg_chunk(B[:, 512:1024], 2048, 512, False)
                mtsB = work.tile([C, 1536], F16, tag="mtsB", bufs=1,
                                 name=f"mtsB_{it}")
                nc.scalar.activation(mtsB[:, 0:1164], B[:, 0:1164], AF.Identity,
                                     bias=gcol[:])
                qpB = ps.tile([C, 1536], F32, tag="big3", bufs=2,
                              name=f"qp_{it}_1")
                q_half(qpB, h_in, mtsB, 1, 1536)
                nc.scalar.activation(h_out[:, 1536:HW], qpB[:, 0:HW - 1536],
                                     AF.Prelu, bias=bia[:, 2:3], alpha=qa)
                if pad is not None:
                    nc.scalar.activation(
                        pad[:, 26:H + 1, 1:W + 1],
                        h_out[:, 1500:HW].rearrange("p (h w) -> p h w", w=W),
                        AF.Copy)
                return h_out

            h1 = agg_q(0, h0)
            gmlp_t(h1, 1)
            h2 = agg_q(1, h1, pad=pad1)

            oc = work.tile([C, 2720], F32, tag="bigf32", bufs=1, name="oc")
            for ri, (r0, nr) in enumerate(RCHUNKS):
                cpe = ps.tile([C, 512], F32, tag="sm", bufs=2, name=f"cp2_{ri}")
                for ti, (dy, dx) in enumerate(taps):
                    idx = (dy * 3 + dx) * 2 + 1
                    nc.tensor.matmul(cpe[:, 0:nr * W], cw[:, idx, :],
                                     pad1[:, r0 + dy:r0 + dy + nr, dx:dx + W],
                                     start=(ti == 0), stop=(ti == 8))
                nc.vector.tensor_tensor(oc[:, r0 * W:(r0 + nr) * W],
                                        cpe[:, 0:nr * W],
                                        convacc[:, r0 * W:(r0 + nr) * W], ALU.add)
                if ri == 2:
                    nc.sync.dma_start(out_d[:, 0:1440], oc[:, 0:1440])
                elif ri == 5:
                    nc.sync.dma_start(out_d[:, 1440:2700], oc[:, 1440:2700])

    nc.compile()
    return nc


def kernel(cnn_encoder_output, original_input, xy,
           g_w0, g_b0, g_a0, g_w1, g_b1, g_a1,
           q_w, q_b, q_a, conv_w, conv_b,
           gnn_iterations, k, use_half_precision, _trace=False):
    assert int(gnn_iterations) == 2 and int(k) == 16 and int(use_half_precision) == 0

    cnn = np.asarray(cnn_encoder_output, dtype=np.float32)
    orig = np.asarray(original_input, dtype=np.float32)
    xy = np.asarray(xy, dtype=np.float32)
    a0, a1, qa = float(np.ravel(g_a0)[0]), float(np.ravel(g_a1)[0]), float(np.ravel(q_a)[0])

    key = (a0, a1, qa)
    if key not in _cache:
        _cache[key] = _build(a0, a1, qa)
    nc = _cache[key]

    g_w0 = np.asarray(g_w0, np.float32)
    g_w1 = np.asarray(g_w1, np.float32)
    q_w = np.asarray(q_w, np.float32)
    conv_w = np.asarray(conv_w, np.float32)

    gw0T = np.ascontiguousarray(g_w0.T).astype(np.float16)
    gw1T = np.ascontiguousarray(g_w1.T).astype(np.float16)
    qw1T = np.ascontiguousarray(q_w[:, :C].T).astype(np.float16)
    qw2T = np.ascontiguousarray(q_w[:, C:].T / float(2 * K)).astype(np.float16)
    cwT = np.empty((C, 18, C), np.float16)
    for dy in range(3):
        for dx in range(3):
            for kh in range(2):
                idx = (dy * 3 + dx) * 2 + kh
                cwT[:, idx, :] = conv_w[:, kh * C:(kh + 1) * C, dy, dx].T.astype(np.float16)
    biases = np.stack([np.asarray(g_b0, np.float32), np.asarray(g_b1, np.float32),
                       np.asarray(q_b, np.float32), np.asarray(conv_b, np.float32)],
                      axis=1)
    b1row = np.stack([np.asarray(g_b1, np.float16),
                      np.asarray(g_b0, np.float16)], axis=0)
    ident = np.eye(C, dtype=np.float16)
    uvc = np.zeros((2, 8, 2816), np.float16)
    uvc[0, 3] = 1.0

    shared = dict(gw0T=gw0T, gw1T=gw1T, qw1T=qw1T, qw2T=qw2T, convwT=cwT,
                  biases=np.ascontiguousarray(biases),
                  b1row=np.ascontiguousarray(b1row), ident=ident, uvc=uvc)
    in_maps = []
    for n in range(N):
        chans = np.stack([xy[n, 0], xy[n, 1], orig[n, 3]], axis=0)      # [3, 360, 480]
        blocks = chans.reshape(3, H, 8, W, 8).transpose(0, 1, 3, 2, 4).reshape(3 * HW, 64)
        blocks = (-blocks).astype(np.float16)
        pad = np.zeros((8192, 64), np.float16)
        pad[:3 * HW] = blocks
        psrcb = pad.reshape(16, 4, 128, 64).transpose(0, 2, 1, 3)
        in_maps.append(dict(h0=np.ascontiguousarray(
                                cnn[n].reshape(C, HW).astype(np.float16)),
                            psrcb=np.ascontiguousarray(psrcb), **shared))

    if _trace:
        _ensure_ntff_hook()
    res = run_bass_kernel_spmd(nc, in_maps, core_ids=list(range(N)), trace=_trace,
                               trace_cores=list(range(N)) if _trace else None)
    out = np.stack([res.results[n]["out"].reshape(C, H, W).astype(np.float32)
                    for n in range(N)])
    if _trace:
        kernel._last_results = res
    return out


# revision 42
# speedup vs baseline: 1.1651x; 1.0336x over previous
"""Trainium2 Bass kernel for EnetGnn (gnn_message_passing).

Data-parallel over batch N=8, one sample per NeuronCore. Per-core design:

1. Median pool: host stages negated fp16 blocks in [16, 128, 4, 64] tiles so
   each load is one contiguous 64KB DMA. DVE max8/match_replace rank-32
   rounds; medians flattened via two half PE transposes + DMA so the x
   channel stages while the second half of the median still runs.
2. KNN threshold: e''[i,j] = p_i.p_j - |p_j|^2/2 (same order as the full
   e') via K=4 fp16 matmuls into double-buffered 3-bank psum halves,
   ACT-evacuated to fp16 ef two tiles ahead of the scans. Per-row
   16th-largest te via max8 + is_ge mask removal + max8 (all DVE, ~8.7us
   per 128-row tile; DVE is the saturated critical engine here).
3. S = Sign(ef - te + eps) on the ACT engine with per-row bias, emitted one
   tile behind the DVE scan so the ACT FIFO never blocks the next tile's
   psum evacuation. S tiles land in one [128, 22, 2720] fp8 SBUF tensor
   with a ones-column that accumulates G = sum_j gh_j.
4. All heavy matmul work runs as one dense tail stream to keep the PE at
   its hot clock: aggregation in fp8 DoubleRow pairs (2 k-tiles per
   instruction) with the G column folded into the mts evacuation as a
   per-partition ACT bias; mts in fp16 keeps the q update all-fp16 (fp32
   moving operands stream 4x slower). The g-MLP's second layer is computed
   in transposed orientation straight into ghrm8 (bias via rank-1 matmul),
   so no PE transposes sit between the MLP and the aggregation.
"""
import numpy as np
import concourse.bass as bass
import concourse.bacc as bacc
import concourse.mybir as mybir
import concourse.tile as tile
from concourse.bass_utils import run_bass_kernel_spmd

F32 = mybir.dt.float32
F16 = mybir.dt.float16
F8 = mybir.dt.float8e4
AF = mybir.ActivationFunctionType
ALU = mybir.AluOpType

N, C, H, W = 8, 128, 45, 60
HW = H * W                      # 2700
K = 16
NEG_F16 = -60000.0

CHUNKS6 = [(0, 512), (512, 512), (1024, 512), (1536, 512), (2048, 512), (2560, 140)]
AGG_CHUNKS = [(0, 512), (512, 512), (1024, 512), (1536, 512), (2048, 512), (2560, 141)]
PTILES = [(t * 128, 128) for t in range(21)] + [(2688, 12)]
RCHUNKS = [(0, 8), (8, 8), (16, 8), (24, 8), (32, 8), (40, 5)]

_cache = {}


def _ensure_ntff_hook():
    import sys
    import types
    try:
        from antenv.axon_hooks import get_axon_ntff_profile_hook  # noqa: F401
        return
    except ImportError:
        pass
    try:
        mod = types.ModuleType("antenv.axon_hooks")
        mod._hook = None

        def set_axon_ntff_profile_hook(h):
            mod._hook = h

        def get_axon_ntff_profile_hook():
            return mod._hook

        mod.set_axon_ntff_profile_hook = set_axon_ntff_profile_hook
        mod.get_axon_ntff_profile_hook = get_axon_ntff_profile_hook
        sys.modules["antenv.axon_hooks"] = mod
        import antenv
        antenv.axon_hooks = mod
        from trn_agent_boot.trn_boot import _ntff_profile_via_ctypes
        hook = _ntff_profile_via_ctypes("/opt/axon/libaxon_pjrt.so")
        if hook is not None:
            mod.set_axon_ntff_profile_hook(hook)
    except Exception as e:  # profiling is best-effort
        print(f"ntff hook injection failed: {e}")


def _build(a0, a1, qa):
    nc = bacc.Bacc("TRN2", target_bir_lowering=False, debug=False, num_devices=8)

    h0_d = nc.dram_tensor("h0", (C, HW), F16, kind="ExternalInput")
    psrcb_d = nc.dram_tensor("psrcb", (16, 128, 4, 64), F16, kind="ExternalInput")
    gw0_d = nc.dram_tensor("gw0T", (C, C), F16, kind="ExternalInput")
    gw1_d = nc.dram_tensor("gw1T", (C, C), F16, kind="ExternalInput")
    qw1_d = nc.dram_tensor("qw1T", (C, C), F16, kind="ExternalInput")
    qw2_d = nc.dram_tensor("qw2T", (C, C), F16, kind="ExternalInput")
    cw_d = nc.dram_tensor("convwT", (C, 18, C), F16, kind="ExternalInput")
    bias_d = nc.dram_tensor("biases", (C, 4), F32, kind="ExternalInput")
    b1row_d = nc.dram_tensor("b1row", (2, C), F16, kind="ExternalInput")
    ident_d = nc.dram_tensor("ident", (C, C), F16, kind="ExternalInput")
    uvc_d = nc.dram_tensor("uvc", (2, 8, 2816), F16, kind="ExternalInput")
    out_d = nc.dram_tensor("out", (C, HW), F32, kind="ExternalOutput")

    with tile.TileContext(nc) as tc:
        with tc.tile_pool(name="sb", bufs=1) as sb, \
             tc.tile_pool(name="work", bufs=2) as work, \
             tc.tile_pool(name="ps", bufs=1, space="PSUM") as ps, \
             tc.tile_pool(name="dram", bufs=1, space="DRAM") as dram:

            projn_d = dram.tile([8192], F16, tag="projn_d")

            # median block DMAs first so the DVE phase starts immediately
            blks = []
            for g in range(16):
                blk = work.tile([128, 4, 64], F16, tag="blk", bufs=8,
                                name=f"blk_{g}")
                nc.sync.dma_start(blk[:], psrcb_d[g])
                blks.append(blk)

            # ---------------- persistent SBUF ----------------
            h0 = sb.tile([C, 2720], F16, tag="h0")
            nc.sync.dma_start(h0[:, 0:HW], h0_d[:])
            gw0 = sb.tile([C, C], F16, tag="gw0")
            nc.sync.dma_start(gw0[:], gw0_d[:])
            gw1 = sb.tile([C, C], F16, tag="gw1")
            nc.sync.dma_start(gw1[:], gw1_d[:])
            qw1 = sb.tile([C, C], F16, tag="qw1")
            nc.sync.dma_start(qw1[:], qw1_d[:])
            qw2 = sb.tile([C, C], F16, tag="qw2")
            nc.sync.dma_start(qw2[:], qw2_d[:])
            cw = sb.tile([C, 18, C], F16, tag="cw")
            nc.sync.dma_start(cw[:], cw_d[:])
            bia = sb.tile([C, 4], F32, tag="bias")
            nc.sync.dma_start(bia[:], bias_d[:])
            b1row = sb.tile([2, C], F16, tag="b1row")
            nc.sync.dma_start(b1row[:], b1row_d[:])
            ones1 = sb.tile([1, C], F16, tag="ones1")
            ident = sb.tile([C, C], F16, tag="ident")
            nc.sync.dma_start(ident[:], ident_d[:])

            U = sb.tile([8, 2816], F16, tag="U")       # [q; 1]
            nc.sync.dma_start(U[:], uvc_d[0])
            V = sb.tile([8, 2816], F16, tag="V")       # [q; -|p|^2/2]
            nc.sync.dma_start(V[:], uvc_d[1])
            Sbig = sb.tile([C, 22, 2720], F8, tag="Sbig")
            ghrm8 = sb.tile([C, 22, 128], F8, tag="ghrm8")
            M8 = sb.tile([C, 64, 8], F16, tag="M8")
            Mt = sb.tile([64, C], F16, tag="Mt")
            pad0 = sb.tile([C, H + 2, W + 2], F16, tag="pad0")
            pad1 = sb.tile([C, H + 2, W + 2], F16, tag="pad1")
            convacc = sb.tile([C, 2720], F32, tag="convacc")
            ones3 = sb.tile([3, 1], F16, tag="ones3")

            # memsets on gpsimd (DVE stays on the median path)
            nc.gpsimd.memset(Sbig[:, :, HW:HW + 1], 1.0)   # ones-cols for G
            nc.gpsimd.memset(pad0[:], 0.0)
            nc.gpsimd.memset(pad1[:], 0.0)
            nc.gpsimd.memset(ones3[:], 1.0)
            nc.gpsimd.memset(ones1[:], 1.0)

            # ---------------- median pooling + split flatten -----------------
            def median_range(glo, ghi):
                for g in range(glo, ghi):
                    blk = blks[g]
                    for s in range(4):
                        mm8 = work.tile([128, 8], F16, tag="mm8", bufs=8)
                        for rnd in range(3):
                            nc.vector.max(mm8[:], blk[:, s, :])
                            nc.vector.match_replace(blk[:, s, :], mm8[:],
                                                    blk[:, s, :], NEG_F16)
                        nc.vector.max(M8[:, g * 4 + s, :], blk[:, s, :])

            def flatten_half(half):
                lo, nc_ = (0, 32) if half == 0 else (32, 32)
                mtp = ps.tile([C, 1024], F16, tag="sm", bufs=2, name=f"mtp{half}")
                Mcols = M8[:, lo:lo + 32, 7:8].rearrange("p a b -> p (a b)")
                nc.tensor.transpose(mtp[0:32, 0:128], Mcols, ident[:])
                nc.scalar.activation(Mt[lo:lo + 32, :], mtp[0:32, 0:128], AF.Copy)
                projn_r = projn_d.rearrange("(a b) -> a b", b=128)
                nc.sync.dma_start(projn_r[lo:lo + 32, :], Mt[lo:lo + 32, :])

            median_range(0, 8)
            median_range(8, 16)

            # ---------------- iter-1 g-MLP + conv h0-half (under median) -----
            def mlp_layer(w, h_in, out, it, lab, bias, alpha):
                for half, o0, on in ((0, 0, 1536), (1, 1536, HW - 1536)):
                    gp = ps.tile([C, 1536], F32, tag="big3", bufs=2,
                                 name=f"{lab}_{it}_{half}")
                    for c0, ncn in (CHUNKS6[:3] if half == 0 else CHUNKS6[3:]):
                        nc.tensor.matmul(gp[:, c0 - o0:c0 - o0 + ncn], w[:],
                                         h_in[:, c0:c0 + ncn], start=True, stop=True)
                    nc.scalar.activation(out[:, o0:o0 + on], gp[:, 0:on], AF.Prelu,
                                         bias=bias, alpha=alpha)

            def gmlp_t(h_in, it):
                """g-MLP: layer 1 straight, layer 2 in transposed orientation
                writing gh2^T tiles directly into ghrm8 (fp8). The b1 bias is
                added via a rank-1 matmul (per-feature = free dim here)."""
                gh1 = work.tile([C, 2720], F16, tag="gh", bufs=1, name=f"gh1_{it}")
                mlp_layer(gw0, h_in, gh1, it, "g1", bia[:, 0:1], a0)
                for jt, (j0, nj) in enumerate(PTILES):
                    lp32 = ps.tile([C, 512], F32, tag="sm", bufs=2,
                                   name=f"l2t_{it}_{jt}")
                    nc.tensor.matmul(lp32[0:nj, 0:128], gh1[:, j0:j0 + nj],
                                     gw1[:], start=True, stop=False)
                    nc.tensor.matmul(lp32[0:nj, 0:128], ones1[0:1, 0:nj],
                                     b1row[0:1, :], start=False, stop=True)
                    nc.scalar.activation(ghrm8[0:nj, jt, :], lp32[0:nj, 0:128],
                                         AF.Prelu, alpha=a1)

            gmlp_t(h0, 0)

            nc.scalar.activation(pad0[:, 1:H + 1, 1:W + 1],
                                 h0[:, 0:HW].rearrange("p (h w) -> p h w", h=H), AF.Copy)
            taps = [(a, b) for a in range(3) for b in range(3)]
            for ri, (r0, nr) in enumerate(RCHUNKS):
                cpe = ps.tile([C, 512], F32, tag="sm", bufs=2, name=f"cpe_{ri}")
                for ti, (dy, dx) in enumerate(taps):
                    idx = (dy * 3 + dx) * 2
                    nc.tensor.matmul(cpe[:, 0:nr * W], cw[:, idx, :],
                                     pad0[:, r0 + dy:r0 + dy + nr, dx:dx + W],
                                     start=(ti == 0), stop=(ti == 8))
                nc.scalar.activation(convacc[:, r0 * W:(r0 + nr) * W],
                                     cpe[:, 0:nr * W], AF.Identity, bias=bia[:, 3:4])

            # ---------------- proj flatten + U/V staging ---------------------
            flatten_half(0)
            # x channel DMA overlaps the second median half
            nc.sync.dma_start(V[0:1, 0:HW], projn_d[0:HW])
            sq3 = work.tile([3, 2720], F16, tag="sq3", bufs=1, name="sq3")

            nc.sync.dma_start(U[0:1, 0:HW], projn_d[0:HW])
            flatten_half(1)
            for ch in (1, 2):
                nc.sync.dma_start(V[ch:ch + 1, 0:HW], projn_d[ch * HW:(ch + 1) * HW])
                nc.sync.dma_start(U[ch:ch + 1, 0:HW], projn_d[ch * HW:(ch + 1) * HW])
            # e'' = q.p - |p_j|^2/2 (same order as e' = 2q.p - |p_j|^2)
            nc.vector.tensor_tensor(sq3[0:3, 0:HW], V[0:3, 0:HW], V[0:3, 0:HW],
                                    ALU.mult)
            sqp = ps.tile([C, 1536], F32, tag="big3", bufs=2, name="sqp")
            for c0, ncn in CHUNKS6[:3]:
                nc.tensor.matmul(sqp[0:1, c0:c0 + ncn], ones3[:],
                                 sq3[:, c0:c0 + ncn], start=True, stop=True)
            hirow = work.tile([1, 2816], F16, tag="row", bufs=1, name="hirow")
            nc.scalar.activation(hirow[0:1, 0:1536], sqp[0:1, 0:1536],
                                 AF.Copy, scale=-0.5)
            nc.sync.dma_start(V[3:4, 0:1536], hirow[0:1, 0:1536])
            sqp2 = ps.tile([C, 1536], F32, tag="big3", bufs=2, name="sqp2")
            for c0, ncn in CHUNKS6[3:]:
                nc.tensor.matmul(sqp2[0:1, c0 - 1536:c0 - 1536 + ncn],
                                 ones3[:], sq3[:, c0:c0 + ncn], start=True, stop=True)
            nc.scalar.activation(hirow[0:1, 1536:HW], sqp2[0:1, 0:HW - 1536],
                                 AF.Copy, scale=-0.5)
            nc.sync.dma_start(V[3:4, 1536:HW], hirow[0:1, 1536:HW])

            # ---------------- p1: per-row te + sign, software-pipelined ------
            efs = {}

            def stage_ef(jt):
                i0, ni = PTILES[jt]
                ef = work.tile([C, 2720], F16, tag="ef", bufs=4, name=f"ef_{jt}")
                efs[jt] = ef
                for half, o0, on in ((0, 0, 1536), (1, 1536, HW - 1536)):
                    pp = ps.tile([C, 1536], F32, tag="big3", bufs=2,
                                 name=f"pp_{jt}_{half}")
                    for c0, ncn in (CHUNKS6[:3] if half == 0 else CHUNKS6[3:]):
                        nc.tensor.matmul(pp[0:ni, c0 - o0:c0 - o0 + ncn],
                                         U[0:4, i0:i0 + ni], V[0:4, c0:c0 + ncn],
                                         start=True, stop=True)
                    nc.scalar.activation(ef[0:ni, o0:o0 + on], pp[0:ni, 0:on],
                                         AF.Copy)

            def p1_scan(jt):
                i0, ni = PTILES[jt]
                ef = efs[jt]
                t8a = work.tile([C, 8], F16, tag="t8", bufs=6, name=f"t8a_{jt}")
                nc.vector.max(t8a[0:ni], ef[0:ni, 0:HW])
                v8f = work.tile([C, 1], F32, tag="v8f", bufs=12, name=f"v8f_{jt}")
                nc.vector.tensor_copy(v8f[0:ni], t8a[0:ni, 7:8])
                msk = work.tile([C, 2720], F16, tag="msk", bufs=3, name=f"msk_{jt}")
                nc.vector.tensor_scalar(msk[0:ni, 0:HW], ef[0:ni, 0:HW],
                                        v8f[0:ni], NEG_F16,
                                        op0=ALU.is_ge, op1=ALU.mult)
                nc.vector.tensor_tensor(msk[0:ni, 0:HW], ef[0:ni, 0:HW],
                                        msk[0:ni, 0:HW], ALU.add)
                t8b = work.tile([C, 8], F16, tag="t8", bufs=6, name=f"t8b_{jt}")
                nc.vector.max(t8b[0:ni], msk[0:ni, 0:HW])
                # bias = -te + |te|*2^-11 + 4e-7
                tp1 = work.tile([C, 1], F32, tag="v8f", bufs=12, name=f"tp1_{jt}")
                nc.vector.tensor_scalar(tp1[0:ni], t8b[0:ni, 7:8], 2.0 ** -11, 0.0,
                                        op0=ALU.mult, op1=ALU.add)
                tab = work.tile([C, 1], F32, tag="v8f", bufs=12, name=f"tab_{jt}")
                nc.vector.scalar_tensor_tensor(tab[0:ni], t8b[0:ni, 7:8],
                                               -(2.0 ** -11), tp1[0:ni],
                                               ALU.mult, ALU.max)
                bv = work.tile([C, 1], F32, tag="v8f", bufs=12, name=f"bv_{jt}")
                nc.vector.scalar_tensor_tensor(bv[0:ni], tab[0:ni], 4.0e-7,
                                               t8b[0:ni, 7:8], ALU.add, ALU.subtract)
                return bv

            def p1_sign(jt, bv):
                i0, ni = PTILES[jt]
                nc.scalar.activation(Sbig[0:ni, jt, 0:HW], efs[jt][0:ni, 0:HW],
                                     AF.Sign, bias=bv[0:ni])

            DR = mybir.MatmulPerfMode.DoubleRow
            # fused agg+q chunk order: G chunk first, then A half, then B
            AGG_ORDER = [(2560, 141, "B", 1024), (0, 512, "A", 0),
                         (512, 512, "A", 512), (1024, 512, "A", 1024),
                         (1536, 512, "B", 0), (2048, 512, "B", 512)]

            def agg_pairs(A, B, c0, ncn, half, off):
                # pairs of full 128-row tiles via fp8 DoubleRow
                tgt = (A if half == "A" else B)[:, off:off + ncn]
                for pr in range(10):
                    nc.tensor.matmul(tgt,
                                     ghrm8[:, 2 * pr:2 * pr + 2, :],
                                     Sbig[:, 2 * pr:2 * pr + 2, c0:c0 + ncn],
                                     start=(pr == 0), stop=False,
                                     perf_mode=DR)

            def agg_tail2(A, B, c0, ncn, half, off):
                # tiles 20 (128 rows) and 21 (12 rows) close each chunk
                tgt = (A if half == "A" else B)[:, off:off + ncn]
                for jt in (20, 21):
                    j0, nj = PTILES[jt]
                    nc.tensor.matmul(tgt,
                                     ghrm8[0:nj, jt, :],
                                     Sbig[0:nj, jt, c0:c0 + ncn],
                                     start=False, stop=(jt == 21))

            def agg_chunk(A, B, c0, ncn, half, off):
                agg_pairs(A, B, c0, ncn, half, off)
                agg_tail2(A, B, c0, ncn, half, off)

            # agg-1 pair accumulation starts during p1's last scans: stage_ef
            # runs 2 tiles ahead, so the big3 psum slots are free and S tiles
            # 0..19 are signed while scans 20/21 still run on the DVE.
            aggAB1 = []
            stage_ef(0)
            stage_ef(1)
            for jt in range(22):
                bv = p1_scan(jt)
                if jt + 2 < 22:
                    stage_ef(jt + 2)
                p1_sign(jt, bv)
                if jt == 19:
                    A1 = ps.tile([C, 1536], F32, tag="big3", bufs=2, name="agg0A")
                    B1 = ps.tile([C, 1536], F32, tag="big3", bufs=2, name="agg0B")
                    aggAB1 = [A1, B1]
                    for c0, ncn, half, off in AGG_ORDER:
                        agg_pairs(A1, B1, c0, ncn, half, off)

            # ---------------- dense tail: agg1+q1, gmlp2, agg2+q2, conv ------

            def q_half(qp, h_in, mts, half, o0):
                for c0, ncn in (CHUNKS6[:3] if half == 0 else CHUNKS6[3:]):
                    nc.tensor.matmul(qp[:, c0 - o0:c0 - o0 + ncn], qw1[:],
                                     h_in[:, c0:c0 + ncn], start=True, stop=False)
                    nc.tensor.matmul(qp[:, c0 - o0:c0 - o0 + ncn], qw2[:],
                                     mts[:, c0 - o0:c0 - o0 + ncn],
                                     start=False, stop=True)

            def agg_q(it, h_in, pad=None, AB=None):
                """Aggregation (G-chunk first) fused with the q update so the
                q matmuls overlap the remaining aggregation chunks. With AB
                given, the DoubleRow pair accumulation already ran during p1
                and only tiles 20/21 close each chunk here."""
                if AB is not None:
                    A, B = AB
                    emit = agg_tail2
                else:
                    A = ps.tile([C, 1536], F32, tag="big3", bufs=2,
                                name=f"agg{it}A")
                    B = ps.tile([C, 1536], F32, tag="big3", bufs=2,
                                name=f"agg{it}B")
                    emit = agg_chunk
                emit(A, B, 2560, 141, "B", 1024)
                gcol = sb.tile([C, 1], F32, tag=f"gcol_{it}")
                nc.scalar.activation(gcol[:], B[:, 1164:1165], AF.Copy)
                for c0 in (0, 512, 1024):
                    emit(A, B, c0, 512, "A", c0)
                mtsA = work.tile([C, 1536], F16, tag="mtsA", bufs=1,
                                 name=f"mtsA_{it}")
                nc.scalar.activation(mtsA[:], A[:, 0:1536], AF.Identity,
                                     bias=gcol[:])
                emit(A, B, 1536, 512, "B", 0)
                h_out = work.tile([C, 2720], F16, tag="h", bufs=2, name=f"h_{it}")
                qpA = ps.tile([C, 1536], F32, tag="big3", bufs=2,
                              name=f"qp_{it}_0")
                q_half(qpA, h_in, mtsA, 0, 0)
                nc.scalar.activation(h_out[:, 0:1536], qpA[:, 0:1536], AF.Prelu,
                                     bias=bia[:, 2:3], alpha=qa)
                if pad is not None:
                    nc.scalar.activation(
                        pad[:, 1:26, 1:W + 1],
                        h_out[:, 0:1500].rearrange("p (h w) -> p h w", w=W),
                        AF.Copy)
                emit(A, B, 2048, 512, "B", 512)
                mtsB = work.tile([C, 1536], F16, tag="mtsB", bufs=1,
                                 name=f"mtsB_{it}")
                nc.scalar.activation(mtsB[:, 0:1164], B[:, 0:1164], AF.Identity,
                                     bias=gcol[:])
                qpB = ps.tile([C, 1536], F32, tag="big3", bufs=2,
                              name=f"qp_{it}_1")
                q_half(qpB, h_in, mtsB, 1, 1536)
                nc.scalar.activation(h_out[:, 1536:HW], qpB[:, 0:HW - 1536],
                                     AF.Prelu, bias=bia[:, 2:3], alpha=qa)
                if pad is not None:
                    nc.scalar.activation(
                        pad[:, 26:H + 1, 1:W + 1],
                        h_out[:, 1500:HW].rearrange("p (h w) -> p h w", w=W),
                        AF.Copy)
                return h_out

            h1 = agg_q(0, h0, AB=aggAB1)
            gmlp_t(h1, 1)
            h2 = agg_q(1, h1, pad=pad1)

            oc = work.tile([C, 2720], F32, tag="bigf32", bufs=1, name="oc")
            for ri, (r0, nr) in enumerate(RCHUNKS):
                cpe = ps.tile([C, 512], F32, tag="sm", bufs=2, name=f"cp2_{ri}")
                for ti, (dy, dx) in enumerate(taps):
                    idx = (dy * 3 + dx) * 2 + 1
                    nc.tensor.matmul(cpe[:, 0:nr * W], cw[:, idx, :],
                                     pad1[:, r0 + dy:r0 + dy + nr, dx:dx + W],
                                     start=(ti == 0), stop=(ti == 8))
                nc.vector.tensor_tensor(oc[:, r0 * W:(r0 + nr) * W],
                                        cpe[:, 0:nr * W],
                                        convacc[:, r0 * W:(r0 + nr) * W], ALU.add)
                if ri == 2:
                    nc.sync.dma_start(out_d[:, 0:1440], oc[:, 0:1440])
                elif ri == 5:
                    nc.sync.dma_start(out_d[:, 1440:2700], oc[:, 1440:2700])

    nc.compile()
    return nc


def kernel(cnn_encoder_output, original_input, xy,
           g_w0, g_b0, g_a0, g_w1, g_b1, g_a1,
           q_w, q_b, q_a, conv_w, conv_b,
           gnn_iterations, k, use_half_precision, _trace=False):
    assert int(gnn_iterations) == 2 and int(k) == 16 and int(use_half_precision) == 0

    cnn = np.asarray(cnn_encoder_output, dtype=np.float32)
    orig = np.asarray(original_input, dtype=np.float32)
    xy = np.asarray(xy, dtype=np.float32)
    a0, a1, qa = float(np.ravel(g_a0)[0]), float(np.ravel(g_a1)[0]), float(np.ravel(q_a)[0])

    key = (a0, a1, qa)
    if key not in _cache:
        _cache[key] = _build(a0, a1, qa)
    nc = _cache[key]

    g_w0 = np.asarray(g_w0, np.float32)
    g_w1 = np.asarray(g_w1, np.float32)
    q_w = np.asarray(q_w, np.float32)
    conv_w = np.asarray(conv_w, np.float32)

    gw0T = np.ascontiguousarray(g_w0.T).astype(np.float16)
    gw1T = np.ascontiguousarray(g_w1.T).astype(np.float16)
    qw1T = np.ascontiguousarray(q_w[:, :C].T).astype(np.float16)
    qw2T = np.ascontiguousarray(q_w[:, C:].T / float(2 * K)).astype(np.float16)
    cwT = np.empty((C, 18, C), np.float16)
    for dy in range(3):
        for dx in range(3):
            for kh in range(2):
                idx = (dy * 3 + dx) * 2 + kh
                cwT[:, idx, :] = conv_w[:, kh * C:(kh + 1) * C, dy, dx].T.astype(np.float16)
    biases = np.stack([np.asarray(g_b0, np.float32), np.asarray(g_b1, np.float32),
                       np.asarray(q_b, np.float32), np.asarray(conv_b, np.float32)],
                      axis=1)
    b1row = np.stack([np.asarray(g_b1, np.float16),
                      np.asarray(g_b0, np.float16)], axis=0)
    ident = np.eye(C, dtype=np.float16)
    uvc = np.zeros((2, 8, 2816), np.float16)
    uvc[0, 3] = 1.0

    shared = dict(gw0T=gw0T, gw1T=gw1T, qw1T=qw1T, qw2T=qw2T, convwT=cwT,
                  biases=np.ascontiguousarray(biases),
                  b1row=np.ascontiguousarray(b1row), ident=ident, uvc=uvc)
    in_maps = []
    for n in range(N):
        chans = np.stack([xy[n, 0], xy[n, 1], orig[n, 3]], axis=0)      # [3, 360, 480]
        blocks = chans.reshape(3, H, 8, W, 8).transpose(0, 1, 3, 2, 4).reshape(3 * HW, 64)
        blocks = (-blocks).astype(np.float16)
        pad = np.zeros((8192, 64), np.float16)
        pad[:3 * HW] = blocks
        psrcb = pad.reshape(16, 4, 128, 64).transpose(0, 2, 1, 3)
        in_maps.append(dict(h0=np.ascontiguousarray(
                                cnn[n].reshape(C, HW).astype(np.float16)),
                            psrcb=np.ascontiguousarray(psrcb), **shared))

    if _trace:
        _ensure_ntff_hook()
    res = run_bass_kernel_spmd(nc, in_maps, core_ids=list(range(N)), trace=_trace,
                               trace_cores=list(range(N)) if _trace else None)
    out = np.stack([res.results[n]["out"].reshape(C, H, W).astype(np.float32)
                    for n in range(N)])
    if _trace:
        kernel._last_results = res
    return out
